# revision 21
# baseline (speedup 1.0000x reference)
"""Trainium2 Bass kernel for nn_EmergentRiskMetrics.

Contract: kernel(**inputs) takes the FULL unsharded inputs (as produced by
setup_inputs()) and returns the FULL output (shape [8], float32).

Sharding: data-parallel over the time axis. Each of the 8 cores owns 1024
contiguous window starts (plus a 128-row halo) for the two rolling-window
correlation scans; the sign-concordance partial sum and cross-sectional
stds are computed on the owning core and combined as scalars on the host.
The full-T covariance (needed on-device for the eigenvalue iteration) is
replicated: every core re-computes X^T X from bf16 tiles of the full
sequence (~1 us of 128^3 bf16 matmuls + ~2 MB of DMA, overlapped with the
rolling phase). An AllReduce-based variant was measured at ~66 us of pure
collective latency for 64 KB on this runtime — replication is far cheaper.

DMA-trigger serialization dominates small-tensor staging, so the host
packs every fp32 constant (masks, identity, MLP weights, positions, ...)
into ONE [128,1024] tensor, and the bf16 bands + pre-transposed chunk
into ONE [128,1664] tensor; x_full lands via 8 big strided DMAs split
across the two HWDGE queues (sync + scalar).

Windowed sums are banded-matrix matmuls on the tensor engine (bands
pre-scaled by 1/sqrt(w) so the mean-correction q^2 term folds into the
V-mask), u = 1/std via reciprocal_approx_fast + Sqrt, and the whole
rolling path runs in bf16 (validated: all rolling-derived outputs have
orders-of-magnitude margin against bf16 noise; d2 >= 0 holds exactly
because S and P derive from the same bf16 x). V*mask+reduce is fused via
tensor_tensor_reduce.

Top eigenvalue: corr is squared 9 times in bf16 (fp32 PSUM accumulate);
traces at step 6 (normalization) and step 9 give lam = (T9*T6^8)^(1/512)
on the host.

Device outputs are per-core partial scalars; the host only gathers them
(sums partial sums, applies the final scalar clips/divides) to assemble
the 8 outputs.
"""

import numpy as np

T = 8192
A = 128
W20 = 20
W10 = 10
NC_N = 8
CHUNK = 1024            # window starts per core
XROWS = 1152            # rows of per-core chunk (9 x 128, incl. halo)
NBLK = XROWS // 128     # 9
R20 = 128 + W20 - 1     # 147
R10 = 128 + W10 - 1     # 137
N20 = T - W20           # 8172 rolling-20 windows
N10 = T - W10           # 8182 rolling-10 windows
OUT_SLOTS = 24
INV_OD = 1.0 / (A * (A - 1))
# rolling20 > 0.7 in corr units == raw quadratic sum > this
THRESH20 = 0.7 * (A * (A - 1)) + A
# static normalization for the eigenvalue squaring chain (~trace(corr^64));
# only needs to be within ~e+-80 of the true value for fp32/bf16 range
EIG_C = 4.0e6

S_COUNT20, S_HIST10, S_RECENT10, S_CSSUM, S_CSFIRST, S_CSLAST, \
    S_SUMCORR, S_SUMABS, S_TRACE, S_PASUM, S_PAMAX, S_SEV, S_SSQ, \
    S_T6, S_T9 = range(15)

# packed fp32 constant tensor column layout
CP_IDENT = 0
CP_M20 = 128                 # 275 cols
CP_M10 = CP_M20 + R20 + 128  # 403, 265 cols
CP_V20 = CP_M10 + R10 + 128  # 668
CP_H10 = CP_V20 + 8
CP_R10 = CP_H10 + 8
CP_W1A = CP_R10 + 8          # 692
CP_W1B = CP_W1A + 128        # 820
CP_B1 = CP_W1B + 128         # 948
CP_GAM = CP_B1 + 1
CP_BET = CP_GAM + 1
CP_W2 = CP_BET + 1           # 951, 64 cols
CP_B2 = CP_W2 + 64           # 1015
CP_W3 = CP_B2 + 1            # 1016, 3 cols
CP_B3 = CP_W3 + 3            # 1019
CP_OH2 = CP_B3 + 1
CP_OH127 = CP_OH2 + 1
CP_POS = CP_OH127 + 1
CP_XLAST = CP_POS + 1
CP_N = CP_XLAST + 1          # 1024

BP_B0 = 0
BP_B1 = 256
BP_XT = 512
BP_N = BP_XT + XROWS         # 1664

_PLAN = {}


def _build_masks():
    # V-masks: +1 over the window band, -1 on the q^2 column
    m20 = np.zeros((128, R20 + 128), np.float32)
    m10 = np.zeros((128, R10 + 128), np.float32)
    for j in range(128):
        m20[j, j:j + W20] = 1.0
        m20[j, R20 + j] = -1.0
        m10[j, j:j + W10] = 1.0
        m10[j, R10 + j] = -1.0
    return m20, m10


def _build_bands():
    # bands0/1 [128 t, 256]: cols 0:128 window-20 (scaled 1/sqrt20),
    # cols 128:256 window-10 (scaled 1/sqrt10). S' = B0^T x_k + B1^T x_{k+1}
    b0 = np.zeros((128, 256), np.float32)
    b1 = np.zeros((128, 256), np.float32)
    s20 = 1.0 / np.sqrt(W20)
    s10 = 1.0 / np.sqrt(W10)
    for j in range(128):
        b0[j:min(128, j + W20), j] = s20
        if j + W20 > 128:
            b1[0:j + W20 - 128, j] = s20
        b0[j:min(128, j + W10), 128 + j] = s10
        if j + W10 > 128:
            b1[0:j + W10 - 128, 128 + j] = s10
    return b0, b1


def _core_masks(c):
    g = c * CHUNK + np.arange(CHUNK)
    valid20 = (g < N20).astype(np.float32)
    hist10 = (g < N10 - 5).astype(np.float32)
    recent10 = ((g >= N10 - 5) & (g < N10)).astype(np.float32)
    # device layout [128 partitions (j in chunk), 8 chunk-columns]
    return (np.ascontiguousarray(valid20.reshape(8, 128).T),
            np.ascontiguousarray(hist10.reshape(8, 128).T),
            np.ascontiguousarray(recent10.reshape(8, 128).T))


def _build_program():
    import os
    import concourse.bacc as bacc
    import concourse.tile as tile
    from concourse import mybir

    kbits = int(os.environ.get("KBITS", "63"))
    bigdma = int(os.environ.get("BIGDMA", "1"))
    # tensor_tensor_reduce hard-crashes the exec unit on this runtime
    use_ttr = int(os.environ.get("TTR", "0"))
    DO_ROLL = kbits & 1
    DO_CS = kbits & 2
    DO_COV = kbits & 4
    DO_EIG = kbits & 8
    DO_POS = kbits & 16
    DO_MLP = kbits & 32

    f32 = mybir.dt.float32
    bf16 = mybir.dt.bfloat16
    ALU = mybir.AluOpType
    ACT = mybir.ActivationFunctionType
    AX = mybir.AxisListType

    nc = bacc.Bacc("TRN2", target_bir_lowering=False, debug=False,
                   num_devices=NC_N)

    def din(name, shape, dt=f32):
        return nc.dram_tensor(name, shape, dt, kind="ExternalInput").ap()

    # partition-major layouts (host pre-permuted): col block i of x_full_pm
    # is x[i*128:(i+1)*128, :] with time-on-partitions — plain contiguous
    # DMAs with one descriptor per partition.
    x_full_pm = din("x_full_pm", [128, 64 * 128], bf16)
    xchunk_pm = din("xchunk_pm", [128, XROWS], bf16)
    cpack_in = din("cpack", [128, CP_N])
    bpack_in = din("bpack", [128, BP_N], bf16)
    out_d = nc.dram_tensor("out_vec", [1, OUT_SLOTS], f32,
                           kind="ExternalOutput").ap()

    with tile.TileContext(nc) as tc:
        with tc.tile_pool(name="const", bufs=1) as cst, \
             tc.tile_pool(name="persist", bufs=1) as per, \
             tc.tile_pool(name="sgs", bufs=3) as sgs, \
             tc.tile_pool(name="wrk", bufs=3) as wrk, \
             tc.tile_pool(name="small", bufs=6) as sml, \
             tc.tile_pool(name="ps", bufs=1, space="PSUM") as ps:

            psum_bufs = {"covq": 1, "band": 2, "zp": 2, "big": 1, "sc": 2}

            def psum(shape, tag):
                return ps.tile(shape, f32, tag=tag, name=tag,
                               bufs=psum_bufs[tag])

            # ---- packed loads: 3 plain DMAs on sync for all staging ----
            bpk = cst.tile([128, BP_N], bf16, tag="bpk")
            nc.sync.dma_start(bpk[:], bpack_in[:, :])
            xck = per.tile([128, XROWS], bf16, tag="xck")
            nc.sync.dma_start(xck[:], xchunk_pm[:, :])
            cpk = cst.tile([128, CP_N], f32, tag="cpk")
            nc.sync.dma_start(cpk[:], cpack_in[:, :])

            b0b = bpk[:, BP_B0:BP_B0 + 256]
            b1b = bpk[:, BP_B1:BP_B1 + 256]
            xTb = bpk[:, BP_XT:BP_XT + XROWS]
            xcbs = [xck[:, j * 128:(j + 1) * 128] for j in range(NBLK)]

            ident = cpk[:, CP_IDENT:CP_IDENT + 128]
            # dedicated mask tiles (vector TENSOR_TENSOR against a slice of
            # the wide packed tile crashed the exec unit)
            m20t = cst.tile([128, R20 + 128], f32, tag="m20t")
            nc.vector.tensor_copy(m20t[:], cpk[:, CP_M20:CP_M20 + R20 + 128])
            m10t = cst.tile([128, R10 + 128], f32, tag="m10t")
            nc.vector.tensor_copy(m10t[:], cpk[:, CP_M10:CP_M10 + R10 + 128])
            m20 = m20t[:]
            m10 = m10t[:]
            v20 = cpk[:, CP_V20:CP_V20 + 8]
            h10 = cpk[:, CP_H10:CP_H10 + 8]
            r10 = cpk[:, CP_R10:CP_R10 + 8]
            w1a = cpk[:, CP_W1A:CP_W1A + 128]
            w1b = cpk[:, CP_W1B:CP_W1B + 128]
            b1 = cpk[:, CP_B1:CP_B1 + 1]
            gam = cpk[:, CP_GAM:CP_GAM + 1]
            bet = cpk[:, CP_BET:CP_BET + 1]
            w2 = cpk[:, CP_W2:CP_W2 + 64]
            b2 = cpk[0:64, CP_B2:CP_B2 + 1]
            w3 = cpk[0:64, CP_W3:CP_W3 + 3]
            b3 = cpk[0:3, CP_B3:CP_B3 + 1]
            oh2 = cpk[0:3, CP_OH2:CP_OH2 + 1]
            oh127 = cpk[:, CP_OH127:CP_OH127 + 1]
            pos_sb = cpk[:, CP_POS:CP_POS + 1]
            xl = cpk[:, CP_XLAST:CP_XLAST + 1]

            ones = cst.tile([128, 1], f32, tag="ones")
            nc.vector.memset(ones[:], 1.0)
            ones_row = cst.tile([1, 128], f32, tag="ones_row")
            nc.vector.memset(ones_row[:], 1.0)

            out_sb = per.tile([1, OUT_SLOTS], f32, tag="out_sb")
            nc.vector.memset(out_sb[:], 0.0)

            def slot(i):
                return out_sb[:, i:i + 1]

            def psum_scalar(vec_sb, p=128):
                o = psum([1, 1], "sc")
                lhs = ones[0:p, :] if p != 128 else ones[:]
                nc.tensor.matmul(o[:], lhsT=lhs, rhs=vec_sb,
                                 start=True, stop=True, skip_group_check=True)
                return o

            # ---- full x for replicated cov: 2 halves on the 2 HWDGE queues
            xfp = per.tile([128, 64 * 128], bf16, tag="xfp")
            if DO_COV:
                if bigdma:
                    nc.sync.dma_start(xfp[:, 0:4096], x_full_pm[:, 0:4096])
                    nc.scalar.dma_start(xfp[:, 4096:8192],
                                        x_full_pm[:, 4096:8192])
                else:
                    for i in range(8):
                        eng = nc.sync if i < 4 else nc.scalar
                        eng.dma_start(xfp[:, i * 1024:(i + 1) * 1024],
                                      x_full_pm[:, i * 1024:(i + 1) * 1024])

            # ---- per-tile squares (gpsimd; reads SBUF only) ----
            xsqbs = []
            for j in range(NBLK):
                xsqb = per.tile([128, 128], bf16, tag="xsqb%d" % j)
                nc.gpsimd.tensor_mul(xsqb[:], xcbs[j], xcbs[j])
                xsqbs.append(xsqb)

            # ---- sharded sign concordance ----
            mq = psum([128, 128], "big")
            for i in range(8):
                sg = sgs.tile([128, 128], bf16, tag="sg")
                nc.scalar.activation(sg[:], xcbs[i], ACT.Sign)
                nc.tensor.matmul(mq[:], lhsT=sg[:], rhs=sg[:],
                                 start=(i == 0), stop=(i == 7),
                                 skip_group_check=True)
            mr = sml.tile([128, 1], f32, tag="mr")
            nc.vector.tensor_reduce(mr[:], mq[:], axis=AX.X, op=ALU.add)
            nc.vector.tensor_copy(slot(S_SSQ), psum_scalar(mr[:])[:])

            # ---- cross-sectional sums (independent; fills startup) ----
            if DO_CS:
                cs_s = per.tile([128, 8], f32, tag="cs_s")
                cs_q = per.tile([128, 8], f32, tag="cs_q")
                for b in range(8):
                    nc.vector.tensor_reduce(cs_s[:, b:b + 1], xcbs[b],
                                            axis=AX.X, op=ALU.add)
                    nc.vector.tensor_reduce(cs_q[:, b:b + 1], xsqbs[b][:],
                                            axis=AX.X, op=ALU.add)

            # ================= position diversity =================
            if DO_POS:
                pa = per.tile([128, 1], f32, tag="pa")
                nc.scalar.activation(pa[:], pos_sb, ACT.Abs)
                nc.vector.tensor_copy(slot(S_PASUM), psum_scalar(pa[:])[:])
                paT_p = psum([1, 128], "sc")
                nc.tensor.transpose(paT_p[:], pa[:], ident)
                paT = sml.tile([1, 128], f32, tag="paT")
                nc.vector.tensor_copy(paT[:], paT_p[:])
                nc.vector.tensor_reduce(slot(S_PAMAX), paT[:], axis=AX.X,
                                        op=ALU.max)

            # ================= herding MLP =================
            if DO_MLP:
                h1p = psum([128, 1], "sc")
                nc.tensor.matmul(h1p[:], lhsT=w1a, rhs=xl, start=True,
                                 stop=False, skip_group_check=True)
                nc.tensor.matmul(h1p[:], lhsT=w1b, rhs=pos_sb,
                                 start=False, stop=True,
                                 skip_group_check=True)
                h1 = sml.tile([128, 1], f32, tag="h1")
                nc.scalar.activation(h1[:], h1p[:], ACT.Relu, bias=b1)
                gk = sml.tile([128, 1], f32, tag="gk")
                nc.vector.tensor_scalar(gk[:], gam,
                                        float(1.0 / np.sqrt(1.0 + 1e-5)),
                                        None, ALU.mult)
                h1b = sml.tile([128, 1], f32, tag="h1b")
                nc.vector.tensor_scalar(h1b[:], h1[:], gk[:], bet,
                                        ALU.mult, ALU.add)
                h2p = psum([64, 1], "sc")
                nc.tensor.matmul(h2p[:], lhsT=w2, rhs=h1b[:], start=True,
                                 stop=True, skip_group_check=True)
                h2 = sml.tile([64, 1], f32, tag="h2")
                nc.scalar.activation(h2[:], h2p[:], ACT.Relu, bias=b2)
                lg = psum([3, 1], "sc")
                nc.tensor.matmul(lg[:], lhsT=w3, rhs=h2[:], start=True,
                                 stop=True, skip_group_check=True)
                exps = sml.tile([3, 1], f32, tag="exps")
                nc.scalar.activation(exps[:], lg[:], ACT.Exp, bias=b3)
                esum = psum_scalar(exps[:], p=3)
                esum_sb = sml.tile([1, 1], f32, tag="esum_sb")
                nc.vector.tensor_copy(esum_sb[:], esum[:])
                erec = sml.tile([1, 1], f32, tag="erec")
                nc.vector.reciprocal(erec[:], esum_sb[:])
                e2p = psum([1, 1], "sc")
                nc.tensor.matmul(e2p[:], lhsT=oh2, rhs=exps[:], start=True,
                                 stop=True, skip_group_check=True)
                e2_sb = sml.tile([1, 1], f32, tag="e2_sb")
                nc.vector.tensor_copy(e2_sb[:], e2p[:])
                nc.vector.tensor_mul(slot(S_SEV), e2_sb[:], erec[:])

            # ---- cov post + eig emitted as closures, woven into the loop ----
            eig_state = {}

            def cov_post():
                cov = per.tile([128, 128], f32, tag="cov")
                nc.scalar.activation(cov[:], covq[:], ACT.Copy)
                dscr = wrk.tile([128, 128], f32, tag="dscr")
                nc.vector.tensor_mul(dscr[:], cov[:], ident)
                diag = per.tile([128, 1], f32, tag="diag")
                nc.vector.tensor_reduce(diag[:], dscr[:], axis=AX.X,
                                        op=ALU.add)
                dstd = per.tile([128, 1], f32, tag="dstd")
                nc.scalar.activation(dstd[:], diag[:], ACT.Sqrt)
                ucol = per.tile([128, 1], f32, tag="ucol")
                nc.vector.reciprocal(ucol[:], dstd[:])
                u2 = sml.tile([128, 1], f32, tag="u2")
                nc.vector.tensor_mul(u2[:], ucol[:], ucol[:])
                du2 = sml.tile([128, 1], f32, tag="du2")
                nc.vector.tensor_mul(du2[:], u2[:], diag[:])
                nc.vector.tensor_copy(slot(S_TRACE), psum_scalar(du2[:])[:])

                uT_p = psum([1, 128], "sc")
                nc.tensor.transpose(uT_p[:], ucol[:], ident)
                uT = per.tile([1, 128], f32, tag="uT")
                nc.vector.tensor_copy(uT[:], uT_p[:])

                def quad_form(mat_sb, out_slot):
                    qr = psum([1, 128], "sc")
                    nc.tensor.matmul(qr[:], lhsT=ucol[:], rhs=mat_sb,
                                     start=True, stop=True,
                                     skip_group_check=True)
                    qscr = sml.tile([1, 128], f32, tag="qscr")
                    nc.vector.tensor_mul(qscr[:], qr[:], uT[:])
                    qacc = sml.tile([1, 1], f32, tag="qacc")
                    nc.vector.tensor_reduce(qacc[:], qscr[:], axis=AX.X,
                                            op=ALU.add)
                    nc.vector.tensor_copy(out_slot, qacc[:])

                quad_form(cov[:], slot(S_SUMCORR))
                acov = per.tile([128, 128], f32, tag="acov")
                nc.scalar.activation(acov[:], cov[:], ACT.Abs)
                quad_form(acov[:], slot(S_SUMABS))

                # corr = diag(u) cov diag(u) -> bf16
                brow = per.tile([128, 128], f32, tag="brow")
                nc.vector.tensor_scalar(brow[:], cov[:], ucol[:], None,
                                        ALU.mult)
                bt_p = psum([128, 128], "big")
                nc.tensor.transpose(bt_p[:], brow[:], ident)
                corr = per.tile([128, 128], bf16, tag="corr")
                nc.scalar.activation(corr[:], bt_p[:], ACT.Copy,
                                     scale=ucol[:])
                eig_state["M"] = corr

            def eig_steps(lo, hi):
                # squaring steps lo..hi-1; static 1/EIG_C normalization at
                # step 5; trace of corr^512 at step 8
                M = eig_state["M"]
                for kk in range(lo, hi):
                    p = psum([128, 128], "big")
                    nc.tensor.matmul(p[:], lhsT=M[:], rhs=M[:],
                                     start=True, stop=True,
                                     skip_group_check=True)
                    Mn = wrk.tile([128, 128], bf16, tag="Mn")
                    if kk == 8:
                        escr = wrk.tile([128, 128], f32, tag="escr")
                        nc.vector.tensor_mul(escr[:], p[:], ident)
                        edg = sml.tile([128, 1], f32, tag="edg")
                        nc.vector.tensor_reduce(edg[:], escr[:], axis=AX.X,
                                                op=ALU.add)
                        trp = psum_scalar(edg[:])
                        nc.vector.tensor_copy(slot(S_T9), trp[:])
                        break
                    nc.scalar.activation(Mn[:], p[:], ACT.Copy,
                                         scale=(1.0 / EIG_C if kk == 5
                                                else 1.0))
                    M = Mn
                eig_state["M"] = M

            # ====== rolling windows + cov + eig chain, interleaved ======
            covq = psum([128, 128], "covq")
            num20 = per.tile([128, 8], f32, tag="num20")
            num10 = per.tile([128, 8], f32, tag="num10")
            for k in range(8):
                if DO_ROLL:
                    sp_ = psum([128, 256], "band")
                    nc.tensor.matmul(sp_[:], lhsT=xcbs[k], rhs=b0b,
                                     start=True, stop=False,
                                     skip_group_check=True)
                    nc.tensor.matmul(sp_[:], lhsT=xcbs[k + 1], rhs=b1b,
                                     start=False, stop=True,
                                     skip_group_check=True)
                    pp = psum([128, 256], "band")
                    nc.tensor.matmul(pp[:], lhsT=xsqbs[k][:], rhs=b0b,
                                     start=True, stop=False,
                                     skip_group_check=True)
                    nc.tensor.matmul(pp[:], lhsT=xsqbs[k + 1][:], rhs=b1b,
                                     start=False, stop=True,
                                     skip_group_check=True)
                    # d2 = P - S^2/w = pp*sqrt(w) - S'^2 (>=0 exactly)
                    sq = wrk.tile([128, 256], f32, tag="sq")
                    nc.scalar.activation(sq[:], sp_[:], ACT.Square)
                    d2 = wrk.tile([128, 256], f32, tag="d2")
                    nc.vector.scalar_tensor_tensor(
                        d2[:, 0:128], in0=pp[:, 0:128],
                        scalar=float(np.sqrt(W20)),
                        in1=sq[:, 0:128], op0=ALU.mult, op1=ALU.subtract)
                    nc.vector.scalar_tensor_tensor(
                        d2[:, 128:256], in0=pp[:, 128:256],
                        scalar=float(np.sqrt(W10)),
                        in1=sq[:, 128:256], op0=ALU.mult, op1=ALU.subtract)
                    rd2 = wrk.tile([128, 256], f32, tag="rd2")
                    nc.vector.reciprocal_approx_fast(rd2[:], d2[:])
                    ub = wrk.tile([128, 256], bf16, tag="ub")
                    nc.scalar.activation(ub[:], rd2[:], ACT.Sqrt)
                    spb = wrk.tile([128, 256], bf16, tag="spb")
                    nc.vector.tensor_copy(spb[:], sp_[:])

                    # w20 mask+reduce on vector (fast elementwise), w10 on
                    # gpsimd — balances the two slowest rolling stages
                    for (wi, R, msk, eng) in ((0, R20, m20, nc.vector),
                                              (1, R10, m10, nc.gpsimd)):
                        zp = psum([128, R + 128], "zp")
                        nc.tensor.matmul(
                            zp[:, 0:R], lhsT=ub[:, wi * 128:(wi + 1) * 128],
                            rhs=xTb[:, k * 128:k * 128 + R],
                            start=True, stop=True, skip_group_check=True)
                        nc.tensor.matmul(
                            zp[:, R:R + 128],
                            lhsT=ub[:, wi * 128:(wi + 1) * 128],
                            rhs=spb[:, wi * 128:(wi + 1) * 128],
                            start=True, stop=True, skip_group_check=True)
                        V = wrk.tile([128, R + 128], f32, tag="V%d" % wi)
                        nc.scalar.activation(V[:], zp[:], ACT.Square)
                        dst = num20 if wi == 0 else num10
                        scr = wrk.tile([128, R + 128], f32, tag="scr%d" % wi)
                        eng.tensor_mul(scr[:], V[:], msk)
                        nc.vector.tensor_reduce(dst[:, k:k + 1], scr[:],
                                                axis=AX.X, op=ALU.add)
                # front-load the 64 replicated cov matmuls into chunks 0-3
                # so the eigenvalue chain can start while rolling finishes
                if DO_COV and k < 4:
                    for i in range(16):
                        t_ = xfp[:, (k * 16 + i) * 128:(k * 16 + i + 1) * 128]
                        nc.tensor.matmul(covq[:], lhsT=t_, rhs=t_,
                                         start=(k == 0 and i == 0),
                                         stop=(k == 3 and i == 15),
                                         skip_group_check=True)
                if DO_COV and k == 4:
                    cov_post()
                if DO_COV and DO_EIG and k >= 5:
                    eig_steps(3 * (k - 5), 3 * (k - 4))

            if DO_ROLL:
                # phase locking count: num20 > thresh, masked valid
                cmp = sml.tile([128, 8], f32, tag="cmp")
                nc.vector.tensor_scalar(cmp[:], num20[:], THRESH20, None,
                                        ALU.is_gt)
                cmp2 = sml.tile([128, 8], f32, tag="cmp2")
                nc.gpsimd.tensor_mul(cmp2[:], cmp[:], v20)
                cnt = sml.tile([128, 1], f32, tag="cnt")
                nc.vector.tensor_reduce(cnt[:], cmp2[:], axis=AX.X,
                                        op=ALU.add)
                nc.vector.tensor_copy(slot(S_COUNT20), psum_scalar(cnt[:])[:])
                hv = sml.tile([128, 8], f32, tag="hv")
                nc.gpsimd.tensor_mul(hv[:], num10[:], h10)
                hs = sml.tile([128, 1], f32, tag="hs")
                nc.vector.tensor_reduce(hs[:], hv[:], axis=AX.X, op=ALU.add)
                nc.vector.tensor_copy(slot(S_HIST10), psum_scalar(hs[:])[:])
                rv = sml.tile([128, 8], f32, tag="rv")
                nc.gpsimd.tensor_mul(rv[:], num10[:], r10)
                rs = sml.tile([128, 1], f32, tag="rs")
                nc.vector.tensor_reduce(rs[:], rv[:], axis=AX.X, op=ALU.add)
                nc.vector.tensor_copy(slot(S_RECENT10), psum_scalar(rs[:])[:])

            # ---- cross-sectional std finish ----
            if DO_CS:
                cs_sq = sml.tile([128, 8], f32, tag="cs_sq")
                nc.scalar.activation(cs_sq[:], cs_s[:], ACT.Square)
                cs_var = sml.tile([128, 8], f32, tag="cs_var")
                nc.vector.scalar_tensor_tensor(
                    cs_var[:], in0=cs_sq[:], scalar=-1.0 / A, in1=cs_q[:],
                    op0=ALU.mult, op1=ALU.add)
                csstd = per.tile([128, 8], f32, tag="csstd")
                nc.scalar.activation(csstd[:], cs_var[:], ACT.Sqrt,
                                     scale=1.0 / (A - 1))
                csr = sml.tile([128, 1], f32, tag="csr")
                nc.vector.tensor_reduce(csr[:], csstd[:], axis=AX.X,
                                        op=ALU.add)
                nc.vector.tensor_copy(slot(S_CSSUM), psum_scalar(csr[:])[:])
                nc.vector.tensor_copy(slot(S_CSFIRST), csstd[0:1, 0:1])
                cslast_p = psum([1, 1], "sc")
                nc.tensor.matmul(cslast_p[:], lhsT=oh127, rhs=csstd[:, 7:8],
                                 start=True, stop=True, skip_group_check=True)
                nc.vector.tensor_copy(slot(S_CSLAST), cslast_p[:])

            # ================= write out =================
            nc.sync.dma_start(out_d[:, :], out_sb[:])

    nc.compile()
    return nc


def _prep_in_maps(inputs):
    import ml_dtypes
    bfloat16 = ml_dtypes.bfloat16
    x = np.ascontiguousarray(np.asarray(inputs["returns_sequence"],
                                        dtype=np.float32))
    xb = x.astype(bfloat16)
    m20, m10 = _build_masks()
    b0, b1 = _build_bands()

    cpack = np.zeros((128, CP_N), np.float32)
    cpack[:, CP_IDENT:CP_IDENT + 128] = np.eye(128, dtype=np.float32)
    cpack[:, CP_M20:CP_M20 + R20 + 128] = m20
    cpack[:, CP_M10:CP_M10 + R10 + 128] = m10
    w1 = np.asarray(inputs["w1"], np.float32)
    cpack[:, CP_W1A:CP_W1A + 128] = w1[0:128]
    cpack[:, CP_W1B:CP_W1B + 128] = w1[128:256]
    cpack[:, CP_B1] = np.asarray(inputs["b1"], np.float32)
    cpack[:, CP_GAM] = np.asarray(inputs["gamma"], np.float32)
    cpack[:, CP_BET] = np.asarray(inputs["beta"], np.float32)
    cpack[:, CP_W2:CP_W2 + 64] = np.asarray(inputs["w2"], np.float32)
    cpack[0:64, CP_B2] = np.asarray(inputs["b2"], np.float32)
    cpack[0:64, CP_W3:CP_W3 + 3] = np.asarray(inputs["w3"], np.float32)
    cpack[0:3, CP_B3] = np.asarray(inputs["b3"], np.float32)
    cpack[2, CP_OH2] = 1.0
    cpack[127, CP_OH127] = 1.0
    cpack[:, CP_POS] = np.asarray(inputs["positions"], np.float32)
    cpack[:, CP_XLAST] = x[-1]

    # partition-major full x: col block i is rows [i*128,(i+1)*128)
    xfull_pm = np.ascontiguousarray(
        xb.reshape(64, 128, 128).transpose(1, 0, 2).reshape(128, 64 * 128))

    in_maps = []
    for c in range(NC_N):
        rows = (c * CHUNK + np.arange(XROWS)) % T
        v20, h10, r10 = _core_masks(c)
        cp = cpack.copy()
        cp[:, CP_V20:CP_V20 + 8] = v20
        cp[:, CP_H10:CP_H10 + 8] = h10
        cp[:, CP_R10:CP_R10 + 8] = r10
        xcb = np.ascontiguousarray(xb[rows])
        xchunk_pm = np.ascontiguousarray(
            xcb.reshape(NBLK, 128, 128).transpose(1, 0, 2)
            .reshape(128, XROWS))
        bpack = np.zeros((128, BP_N), bfloat16)
        bpack[:, BP_B0:BP_B0 + 256] = b0.astype(bfloat16)
        bpack[:, BP_B1:BP_B1 + 256] = b1.astype(bfloat16)
        bpack[:, BP_XT:BP_XT + XROWS] = xcb.T
        in_maps.append({
            "x_full_pm": xfull_pm,
            "xchunk_pm": xchunk_pm,
            "cpack": cp,
            "bpack": bpack,
        })
    return in_maps


def _combine(per_core):
    count20 = sum(float(per_core[c][0, S_COUNT20]) for c in range(NC_N))
    hist_raw = sum(float(per_core[c][0, S_HIST10]) for c in range(NC_N))
    rec_raw = sum(float(per_core[c][0, S_RECENT10]) for c in range(NC_N))
    cs_sum = sum(float(per_core[c][0, S_CSSUM]) for c in range(NC_N))
    ssq_sum = sum(float(per_core[c][0, S_SSQ]) for c in range(NC_N))
    cs_first = float(per_core[0][0, S_CSFIRST])
    cs_last = float(per_core[NC_N - 1][0, S_CSLAST])
    r0 = per_core[0][0]
    sum_corr = float(r0[S_SUMCORR])
    sum_abs = float(r0[S_SUMABS])
    trace_c = float(r0[S_TRACE])
    pa_sum = float(r0[S_PASUM])
    pa_max = float(r0[S_PAMAX])
    severity = float(r0[S_SEV])
    T9 = float(r0[S_T9])

    phase_locking = count20 / N20
    nh = N10 - 5
    hist = (hist_raw - nh * A) * INV_OD / nh
    recent = (rec_raw - 5 * A) * INV_OD / 5.0
    surge = 0.0
    if hist > 0:
        surge = min(max((recent - hist) / hist, 0.0), 1.0)
    avg_disp = cs_sum / T
    trend = -(cs_last - cs_first) / (T - 1)
    herding_index = min(max(trend / (avg_disp + 1e-6) + 0.5, 0.0), 1.0)
    avg_corr = (sum_corr - trace_c) / (A * (A - 1))
    # T9 = trace(corr^512) / EIG_C^8 on device; lam ~ trace(corr^512)^(1/512)
    lam = np.exp((8.0 * np.log(EIG_C) + np.log(T9)) / 512.0)
    sync_risk = min(1.0, (lam / A) * avg_corr)
    return_div = 1.0 - sum_abs / (A * A)
    pos_div = 1.0 - pa_max / pa_sum
    div_loss = 1.0 - np.sqrt(return_div * pos_div)
    avg_conc = (A * A / 2.0 + ssq_sum / (2.0 * T) - A) / (A * (A - 1))
    phase_coupling = min(max((avg_conc - 0.5) * 2.0, 0.0), 1.0)
    collective = (herding_index + sync_risk + div_loss) / 3.0
    return np.array([herding_index, severity, sync_risk, phase_locking,
                     div_loss, surge, phase_coupling, collective],
                    dtype=np.float32)


def _ensure_ntff_hook():
    """Install the axon NTFF profile hook if the image lacks antenv.axon_hooks."""
    import sys
    import types
    try:
        import antenv.axon_hooks  # noqa: F401
        return True
    except ImportError:
        pass
    try:
        import antenv
        from trn_agent_boot.trn_boot import _ntff_profile_via_ctypes
        mod = types.ModuleType("antenv.axon_hooks")
        state = {}
        mod.set_axon_ntff_profile_hook = lambda h: state.update(h=h)
        mod.get_axon_ntff_profile_hook = lambda: state.get("h")
        sys.modules["antenv.axon_hooks"] = mod
        antenv.axon_hooks = mod
        hook = _ntff_profile_via_ctypes("/opt/axon/libaxon_pjrt.so")
        mod.set_axon_ntff_profile_hook(hook)
        return hook is not None
    except Exception:
        return False


def _run(inputs, trace=False):
    from concourse.bass_utils import run_bass_kernel_spmd
    if trace:
        trace = _ensure_ntff_hook()
    if "nc" not in _PLAN:
        _PLAN["nc"] = _build_program()
    nc = _PLAN["nc"]
    in_maps = _prep_in_maps(inputs)
    res = run_bass_kernel_spmd(nc, in_maps, core_ids=list(range(NC_N)),
                               trace=trace)
    per_core = [res.results[c]["out_vec"] for c in range(NC_N)]
    return _combine(per_core), res


def kernel(**inputs) -> np.ndarray:
    out, _ = _run(inputs, trace=False)
    return out


# revision 22
# speedup vs baseline: 1.0173x; 1.0173x over previous
"""Trainium2 Bass kernel for nn_EmergentRiskMetrics.

Contract: kernel(**inputs) takes the FULL unsharded inputs (as produced by
setup_inputs()) and returns the FULL output (shape [8], float32).

Sharding: data-parallel over the time axis. Each of the 8 cores owns 1024
contiguous window starts (plus a 128-row halo) for the two rolling-window
correlation scans; the sign-concordance partial sum and cross-sectional
stds are computed on the owning core and combined as scalars on the host.
The full-T covariance (needed on-device for the eigenvalue iteration) is
replicated: every core re-computes X^T X from bf16 tiles of the full
sequence (~1 us of 128^3 bf16 matmuls + ~2 MB of DMA, overlapped with the
rolling phase). An AllReduce-based variant was measured at ~66 us of pure
collective latency for 64 KB on this runtime — replication is far cheaper.

DMA-trigger serialization dominates small-tensor staging, so the host
packs every fp32 constant (masks, identity, MLP weights, positions, ...)
into ONE [128,1024] tensor, and the bf16 bands + pre-transposed chunk
into ONE [128,1664] tensor; x_full lands via 8 big strided DMAs split
across the two HWDGE queues (sync + scalar).

Windowed sums are banded-matrix matmuls on the tensor engine (bands
pre-scaled by 1/sqrt(w) so the mean-correction q^2 term folds into the
V-mask), u = 1/std via reciprocal_approx_fast + Sqrt, and the whole
rolling path runs in bf16 (validated: all rolling-derived outputs have
orders-of-magnitude margin against bf16 noise; d2 >= 0 holds exactly
because S and P derive from the same bf16 x). V*mask+reduce is fused via
tensor_tensor_reduce.

Top eigenvalue: corr is squared 9 times in bf16 (fp32 PSUM accumulate);
traces at step 6 (normalization) and step 9 give lam = (T9*T6^8)^(1/512)
on the host.

Device outputs are per-core partial scalars; the host only gathers them
(sums partial sums, applies the final scalar clips/divides) to assemble
the 8 outputs.
"""

import numpy as np

T = 8192
A = 128
W20 = 20
W10 = 10
NC_N = 8
CHUNK = 1024            # window starts per core
XROWS = 1152            # rows of per-core chunk (9 x 128, incl. halo)
NBLK = XROWS // 128     # 9
R20 = 128 + W20 - 1     # 147
R10 = 128 + W10 - 1     # 137
N20 = T - W20           # 8172 rolling-20 windows
N10 = T - W10           # 8182 rolling-10 windows
OUT_SLOTS = 24
INV_OD = 1.0 / (A * (A - 1))
# rolling20 > 0.7 in corr units == raw quadratic sum > this
THRESH20 = 0.7 * (A * (A - 1)) + A
# static normalization for the eigenvalue squaring chain (~trace(corr^64));
# only needs to be within ~e+-80 of the true value for fp32/bf16 range
EIG_C = 4.0e6

S_COUNT20, S_HIST10, S_RECENT10, S_CSSUM, S_CSFIRST, S_CSLAST, \
    S_SUMCORR, S_SUMABS, S_TRACE, S_PASUM, S_PAMAX, S_SEV, S_SSQ, \
    S_T6, S_T9 = range(15)

# packed fp32 constant tensor column layout
CP_IDENT = 0
CP_M20 = 128                 # 275 cols
CP_M10 = CP_M20 + R20 + 128  # 403, 265 cols
CP_V20 = CP_M10 + R10 + 128  # 668
CP_H10 = CP_V20 + 8
CP_R10 = CP_H10 + 8
CP_W1A = CP_R10 + 8          # 692
CP_W1B = CP_W1A + 128        # 820
CP_B1 = CP_W1B + 128         # 948
CP_GAM = CP_B1 + 1
CP_BET = CP_GAM + 1
CP_W2 = CP_BET + 1           # 951, 64 cols
CP_B2 = CP_W2 + 64           # 1015
CP_W3 = CP_B2 + 1            # 1016, 3 cols
CP_B3 = CP_W3 + 3            # 1019
CP_OH2 = CP_B3 + 1
CP_OH127 = CP_OH2 + 1
CP_POS = CP_OH127 + 1
CP_XLAST = CP_POS + 1
CP_N = CP_XLAST + 1          # 1024

BP_B0 = 0
BP_B1 = 256
BP_XT = 512
BP_N = BP_XT + XROWS         # 1664

_PLAN = {}


def _build_masks():
    # 0/1 window-membership bands (the q^2 term is handled separately)
    m20 = np.zeros((128, R20), np.float32)
    m10 = np.zeros((128, R10), np.float32)
    for j in range(128):
        m20[j, j:j + W20] = 1.0
        m10[j, j:j + W10] = 1.0
    return m20, m10


def _build_bands():
    # bands0/1 [128 t, 256]: cols 0:128 window-20 (scaled 1/sqrt20),
    # cols 128:256 window-10 (scaled 1/sqrt10). S' = B0^T x_k + B1^T x_{k+1}
    b0 = np.zeros((128, 256), np.float32)
    b1 = np.zeros((128, 256), np.float32)
    s20 = 1.0 / np.sqrt(W20)
    s10 = 1.0 / np.sqrt(W10)
    for j in range(128):
        b0[j:min(128, j + W20), j] = s20
        if j + W20 > 128:
            b1[0:j + W20 - 128, j] = s20
        b0[j:min(128, j + W10), 128 + j] = s10
        if j + W10 > 128:
            b1[0:j + W10 - 128, 128 + j] = s10
    return b0, b1


def _core_masks(c):
    g = c * CHUNK + np.arange(CHUNK)
    valid20 = (g < N20).astype(np.float32)
    hist10 = (g < N10 - 5).astype(np.float32)
    recent10 = ((g >= N10 - 5) & (g < N10)).astype(np.float32)
    # device layout [128 partitions (j in chunk), 8 chunk-columns]
    return (np.ascontiguousarray(valid20.reshape(8, 128).T),
            np.ascontiguousarray(hist10.reshape(8, 128).T),
            np.ascontiguousarray(recent10.reshape(8, 128).T))


def _build_program():
    import os
    import concourse.bacc as bacc
    import concourse.tile as tile
    from concourse import mybir

    kbits = int(os.environ.get("KBITS", "63"))
    bigdma = int(os.environ.get("BIGDMA", "1"))
    # tensor_tensor_reduce hard-crashes the exec unit on this runtime
    use_ttr = int(os.environ.get("TTR", "0"))
    DO_ROLL = kbits & 1
    DO_CS = kbits & 2
    DO_COV = kbits & 4
    DO_EIG = kbits & 8
    DO_POS = kbits & 16
    DO_MLP = kbits & 32

    f32 = mybir.dt.float32
    bf16 = mybir.dt.bfloat16
    ALU = mybir.AluOpType
    ACT = mybir.ActivationFunctionType
    AX = mybir.AxisListType

    nc = bacc.Bacc("TRN2", target_bir_lowering=False, debug=False,
                   num_devices=NC_N)

    def din(name, shape, dt=f32):
        return nc.dram_tensor(name, shape, dt, kind="ExternalInput").ap()

    # partition-major layouts (host pre-permuted): col block i of x_full_pm
    # is x[i*128:(i+1)*128, :] with time-on-partitions — plain contiguous
    # DMAs with one descriptor per partition.
    x_full_pm = din("x_full_pm", [128, 64 * 128], bf16)
    xchunk_pm = din("xchunk_pm", [128, XROWS], bf16)
    cpack_in = din("cpack", [128, CP_N])
    bpack_in = din("bpack", [128, BP_N], bf16)
    out_d = nc.dram_tensor("out_vec", [1, OUT_SLOTS], f32,
                           kind="ExternalOutput").ap()

    with tile.TileContext(nc) as tc:
        with tc.tile_pool(name="const", bufs=1) as cst, \
             tc.tile_pool(name="persist", bufs=1) as per, \
             tc.tile_pool(name="sgs", bufs=3) as sgs, \
             tc.tile_pool(name="wrk", bufs=3) as wrk, \
             tc.tile_pool(name="small", bufs=6) as sml, \
             tc.tile_pool(name="ps", bufs=1, space="PSUM") as ps:

            psum_bufs = {"covq": 1, "band": 2, "zp": 2, "big": 1, "sc": 2}

            def psum(shape, tag):
                return ps.tile(shape, f32, tag=tag, name=tag,
                               bufs=psum_bufs[tag])

            # ---- packed loads: bpack/xchunk on sync, cpack on scalar ----
            bpk = cst.tile([128, BP_N], bf16, tag="bpk")
            nc.sync.dma_start(bpk[:], bpack_in[:, :])
            xck = per.tile([128, XROWS], bf16, tag="xck")
            nc.sync.dma_start(xck[:], xchunk_pm[:, :])
            cpk = cst.tile([128, CP_N], f32, tag="cpk")
            nc.scalar.dma_start(cpk[:], cpack_in[:, :])

            b0b = bpk[:, BP_B0:BP_B0 + 256]
            b1b = bpk[:, BP_B1:BP_B1 + 256]
            xTb = bpk[:, BP_XT:BP_XT + XROWS]
            xcbs = [xck[:, j * 128:(j + 1) * 128] for j in range(NBLK)]

            ident = cpk[:, CP_IDENT:CP_IDENT + 128]
            # dedicated mask tiles (vector TENSOR_TENSOR against a slice of
            # the wide packed tile crashed the exec unit)
            m20t = cst.tile([128, R20], f32, tag="m20t")
            nc.vector.tensor_copy(m20t[:], cpk[:, CP_M20:CP_M20 + R20])
            m10t = cst.tile([128, R10], f32, tag="m10t")
            nc.vector.tensor_copy(m10t[:], cpk[:, CP_M10:CP_M10 + R10])
            m20 = m20t[:]
            m10 = m10t[:]
            v20 = cpk[:, CP_V20:CP_V20 + 8]
            h10 = cpk[:, CP_H10:CP_H10 + 8]
            r10 = cpk[:, CP_R10:CP_R10 + 8]
            w1a = cpk[:, CP_W1A:CP_W1A + 128]
            w1b = cpk[:, CP_W1B:CP_W1B + 128]
            b1 = cpk[:, CP_B1:CP_B1 + 1]
            gam = cpk[:, CP_GAM:CP_GAM + 1]
            bet = cpk[:, CP_BET:CP_BET + 1]
            w2 = cpk[:, CP_W2:CP_W2 + 64]
            b2 = cpk[0:64, CP_B2:CP_B2 + 1]
            w3 = cpk[0:64, CP_W3:CP_W3 + 3]
            b3 = cpk[0:3, CP_B3:CP_B3 + 1]
            oh2 = cpk[0:3, CP_OH2:CP_OH2 + 1]
            oh127 = cpk[:, CP_OH127:CP_OH127 + 1]
            pos_sb = cpk[:, CP_POS:CP_POS + 1]
            xl = cpk[:, CP_XLAST:CP_XLAST + 1]

            ones = cst.tile([128, 1], f32, tag="ones")
            nc.vector.memset(ones[:], 1.0)
            ones_row = cst.tile([1, 128], f32, tag="ones_row")
            nc.vector.memset(ones_row[:], 1.0)

            out_sb = per.tile([1, OUT_SLOTS], f32, tag="out_sb")
            nc.vector.memset(out_sb[:], 0.0)

            def slot(i):
                return out_sb[:, i:i + 1]

            def psum_scalar(vec_sb, p=128):
                o = psum([1, 1], "sc")
                lhs = ones[0:p, :] if p != 128 else ones[:]
                nc.tensor.matmul(o[:], lhsT=lhs, rhs=vec_sb,
                                 start=True, stop=True, skip_group_check=True)
                return o

            # ---- full x for replicated cov: 2 halves on the 2 HWDGE queues
            xfp = per.tile([128, 64 * 128], bf16, tag="xfp")
            if DO_COV:
                # 4 quarters alternating queues; cov chunk k consumes
                # quarter k, so the earliest-landing quarters go first
                for i in range(4):
                    eng = nc.sync if i % 2 == 0 else nc.scalar
                    eng.dma_start(xfp[:, i * 2048:(i + 1) * 2048],
                                  x_full_pm[:, i * 2048:(i + 1) * 2048])

            # ---- per-tile squares (gpsimd; reads SBUF only) ----
            xsqbs = []
            for j in range(NBLK):
                xsqb = per.tile([128, 128], bf16, tag="xsqb%d" % j)
                nc.gpsimd.tensor_mul(xsqb[:], xcbs[j], xcbs[j])
                xsqbs.append(xsqb)

            # ---- sharded sign concordance ----
            mq = psum([128, 128], "big")
            for i in range(8):
                sg = sgs.tile([128, 128], bf16, tag="sg")
                nc.scalar.activation(sg[:], xcbs[i], ACT.Sign)
                nc.tensor.matmul(mq[:], lhsT=sg[:], rhs=sg[:],
                                 start=(i == 0), stop=(i == 7),
                                 skip_group_check=True)
            mr = sml.tile([128, 1], f32, tag="mr")
            nc.vector.tensor_reduce(mr[:], mq[:], axis=AX.X, op=ALU.add)
            nc.vector.tensor_copy(slot(S_SSQ), psum_scalar(mr[:])[:])

            # ---- cross-sectional sums (independent; fills startup) ----
            if DO_CS:
                cs_s = per.tile([128, 8], f32, tag="cs_s")
                cs_q = per.tile([128, 8], f32, tag="cs_q")
                for b in range(8):
                    nc.vector.tensor_reduce(cs_s[:, b:b + 1], xcbs[b],
                                            axis=AX.X, op=ALU.add)
                    nc.vector.tensor_reduce(cs_q[:, b:b + 1], xsqbs[b][:],
                                            axis=AX.X, op=ALU.add)

            # ================= position diversity =================
            if DO_POS:
                pa = per.tile([128, 1], f32, tag="pa")
                nc.scalar.activation(pa[:], pos_sb, ACT.Abs)
                nc.vector.tensor_copy(slot(S_PASUM), psum_scalar(pa[:])[:])
                paT_p = psum([1, 128], "sc")
                nc.tensor.transpose(paT_p[:], pa[:], ident)
                paT = sml.tile([1, 128], f32, tag="paT")
                nc.vector.tensor_copy(paT[:], paT_p[:])
                nc.vector.tensor_reduce(slot(S_PAMAX), paT[:], axis=AX.X,
                                        op=ALU.max)

            # ================= herding MLP =================
            if DO_MLP:
                h1p = psum([128, 1], "sc")
                nc.tensor.matmul(h1p[:], lhsT=w1a, rhs=xl, start=True,
                                 stop=False, skip_group_check=True)
                nc.tensor.matmul(h1p[:], lhsT=w1b, rhs=pos_sb,
                                 start=False, stop=True,
                                 skip_group_check=True)
                h1 = sml.tile([128, 1], f32, tag="h1")
                nc.scalar.activation(h1[:], h1p[:], ACT.Relu, bias=b1)
                gk = sml.tile([128, 1], f32, tag="gk")
                nc.vector.tensor_scalar(gk[:], gam,
                                        float(1.0 / np.sqrt(1.0 + 1e-5)),
                                        None, ALU.mult)
                h1b = sml.tile([128, 1], f32, tag="h1b")
                nc.vector.tensor_scalar(h1b[:], h1[:], gk[:], bet,
                                        ALU.mult, ALU.add)
                h2p = psum([64, 1], "sc")
                nc.tensor.matmul(h2p[:], lhsT=w2, rhs=h1b[:], start=True,
                                 stop=True, skip_group_check=True)
                h2 = sml.tile([64, 1], f32, tag="h2")
                nc.scalar.activation(h2[:], h2p[:], ACT.Relu, bias=b2)
                lg = psum([3, 1], "sc")
                nc.tensor.matmul(lg[:], lhsT=w3, rhs=h2[:], start=True,
                                 stop=True, skip_group_check=True)
                exps = sml.tile([3, 1], f32, tag="exps")
                nc.scalar.activation(exps[:], lg[:], ACT.Exp, bias=b3)
                esum = psum_scalar(exps[:], p=3)
                esum_sb = sml.tile([1, 1], f32, tag="esum_sb")
                nc.vector.tensor_copy(esum_sb[:], esum[:])
                erec = sml.tile([1, 1], f32, tag="erec")
                nc.vector.reciprocal(erec[:], esum_sb[:])
                e2p = psum([1, 1], "sc")
                nc.tensor.matmul(e2p[:], lhsT=oh2, rhs=exps[:], start=True,
                                 stop=True, skip_group_check=True)
                e2_sb = sml.tile([1, 1], f32, tag="e2_sb")
                nc.vector.tensor_copy(e2_sb[:], e2p[:])
                nc.vector.tensor_mul(slot(S_SEV), e2_sb[:], erec[:])

            # ---- cov post + eig emitted as closures, woven into the loop ----
            eig_state = {}

            def cov_post():
                cov = per.tile([128, 128], f32, tag="cov")
                nc.scalar.activation(cov[:], covq[:], ACT.Copy)
                dscr = wrk.tile([128, 128], f32, tag="dscr")
                nc.vector.tensor_mul(dscr[:], cov[:], ident)
                diag = per.tile([128, 1], f32, tag="diag")
                nc.vector.tensor_reduce(diag[:], dscr[:], axis=AX.X,
                                        op=ALU.add)
                dstd = per.tile([128, 1], f32, tag="dstd")
                nc.scalar.activation(dstd[:], diag[:], ACT.Sqrt)
                ucol = per.tile([128, 1], f32, tag="ucol")
                nc.vector.reciprocal(ucol[:], dstd[:])
                u2 = sml.tile([128, 1], f32, tag="u2")
                nc.vector.tensor_mul(u2[:], ucol[:], ucol[:])
                du2 = sml.tile([128, 1], f32, tag="du2")
                nc.vector.tensor_mul(du2[:], u2[:], diag[:])
                nc.vector.tensor_copy(slot(S_TRACE), psum_scalar(du2[:])[:])

                uT_p = psum([1, 128], "sc")
                nc.tensor.transpose(uT_p[:], ucol[:], ident)
                uT = per.tile([1, 128], f32, tag="uT")
                nc.vector.tensor_copy(uT[:], uT_p[:])

                def quad_form(mat_sb, out_slot):
                    qr = psum([1, 128], "sc")
                    nc.tensor.matmul(qr[:], lhsT=ucol[:], rhs=mat_sb,
                                     start=True, stop=True,
                                     skip_group_check=True)
                    qscr = sml.tile([1, 128], f32, tag="qscr")
                    nc.vector.tensor_mul(qscr[:], qr[:], uT[:])
                    qacc = sml.tile([1, 1], f32, tag="qacc")
                    nc.vector.tensor_reduce(qacc[:], qscr[:], axis=AX.X,
                                            op=ALU.add)
                    nc.vector.tensor_copy(out_slot, qacc[:])

                quad_form(cov[:], slot(S_SUMCORR))
                acov = per.tile([128, 128], f32, tag="acov")
                nc.scalar.activation(acov[:], cov[:], ACT.Abs)
                quad_form(acov[:], slot(S_SUMABS))

                # corr = diag(u) cov diag(u) -> bf16
                brow = per.tile([128, 128], f32, tag="brow")
                nc.vector.tensor_scalar(brow[:], cov[:], ucol[:], None,
                                        ALU.mult)
                bt_p = psum([128, 128], "big")
                nc.tensor.transpose(bt_p[:], brow[:], ident)
                corr = per.tile([128, 128], bf16, tag="corr")
                nc.scalar.activation(corr[:], bt_p[:], ACT.Copy,
                                     scale=ucol[:])
                eig_state["M"] = corr

            def eig_steps(lo, hi):
                # squaring steps lo..hi-1; static 1/EIG_C normalization at
                # step 5; trace of corr^512 at step 8
                M = eig_state["M"]
                for kk in range(lo, hi):
                    p = psum([128, 128], "big")
                    nc.tensor.matmul(p[:], lhsT=M[:], rhs=M[:],
                                     start=True, stop=True,
                                     skip_group_check=True)
                    Mn = wrk.tile([128, 128], bf16, tag="Mn")
                    if kk == 8:
                        escr = wrk.tile([128, 128], f32, tag="escr")
                        nc.vector.tensor_mul(escr[:], p[:], ident)
                        edg = sml.tile([128, 1], f32, tag="edg")
                        nc.vector.tensor_reduce(edg[:], escr[:], axis=AX.X,
                                                op=ALU.add)
                        trp = psum_scalar(edg[:])
                        nc.vector.tensor_copy(slot(S_T9), trp[:])
                        break
                    nc.scalar.activation(Mn[:], p[:], ACT.Copy,
                                         scale=(1.0 / EIG_C if kk == 5
                                                else 1.0))
                    M = Mn
                eig_state["M"] = M

            # ====== rolling windows + cov + eig chain, interleaved ======
            covq = psum([128, 128], "covq")
            num20 = per.tile([128, 8], f32, tag="num20")
            num10 = per.tile([128, 8], f32, tag="num10")
            qsq20 = per.tile([128, 8], f32, tag="qsq20")
            qsq10 = per.tile([128, 8], f32, tag="qsq10")
            for k in range(8):
                if DO_ROLL:
                    sp_ = psum([128, 256], "band")
                    nc.tensor.matmul(sp_[:], lhsT=xcbs[k], rhs=b0b,
                                     start=True, stop=False,
                                     skip_group_check=True)
                    nc.tensor.matmul(sp_[:], lhsT=xcbs[k + 1], rhs=b1b,
                                     start=False, stop=True,
                                     skip_group_check=True)
                    pp = psum([128, 256], "band")
                    nc.tensor.matmul(pp[:], lhsT=xsqbs[k][:], rhs=b0b,
                                     start=True, stop=False,
                                     skip_group_check=True)
                    nc.tensor.matmul(pp[:], lhsT=xsqbs[k + 1][:], rhs=b1b,
                                     start=False, stop=True,
                                     skip_group_check=True)
                    # d2 = P - S^2/w = pp*sqrt(w) - S'^2 (>=0 exactly)
                    sq = wrk.tile([128, 256], f32, tag="sq")
                    nc.scalar.activation(sq[:], sp_[:], ACT.Square)
                    d2 = wrk.tile([128, 256], f32, tag="d2")
                    nc.vector.scalar_tensor_tensor(
                        d2[:, 0:128], in0=pp[:, 0:128],
                        scalar=float(np.sqrt(W20)),
                        in1=sq[:, 0:128], op0=ALU.mult, op1=ALU.subtract)
                    nc.vector.scalar_tensor_tensor(
                        d2[:, 128:256], in0=pp[:, 128:256],
                        scalar=float(np.sqrt(W10)),
                        in1=sq[:, 128:256], op0=ALU.mult, op1=ALU.subtract)
                    rd2 = wrk.tile([128, 256], f32, tag="rd2")
                    nc.vector.reciprocal_approx_fast(rd2[:], d2[:])
                    ub = wrk.tile([128, 256], bf16, tag="ub")
                    nc.scalar.activation(ub[:], rd2[:], ACT.Sqrt)
                    spb = wrk.tile([128, 256], bf16, tag="spb")
                    nc.vector.tensor_copy(spb[:], sp_[:])

                    # q_j = sum_a u*S' via elementwise mul + ones-matmul
                    us = wrk.tile([128, 256], f32, tag="us")
                    nc.gpsimd.tensor_mul(us[:], ub[:], spb[:])
                    for (wi, R, msk) in ((0, R20, m20), (1, R10, m10)):
                        qp = psum([128, 1], "sc")
                        nc.tensor.matmul(
                            qp[:], lhsT=us[:, wi * 128:(wi + 1) * 128],
                            rhs=ones[:], start=True, stop=True,
                            skip_group_check=True)
                        qdst = qsq20 if wi == 0 else qsq10
                        nc.scalar.activation(qdst[:, k:k + 1], qp[:],
                                             ACT.Square)
                        zp = psum([128, R], "zp")
                        nc.tensor.matmul(
                            zp[:], lhsT=ub[:, wi * 128:(wi + 1) * 128],
                            rhs=xTb[:, k * 128:k * 128 + R],
                            start=True, stop=True, skip_group_check=True)
                        # mask to the window band, then square+row-reduce in
                        # one scalar-engine activation (accum_out)
                        zm = wrk.tile([128, R], f32, tag="zm%d" % wi)
                        nc.vector.tensor_mul(zm[:], zp[:], msk)
                        V = wrk.tile([128, R], f32, tag="V%d" % wi)
                        dst = num20 if wi == 0 else num10
                        nc.scalar.activation(V[:], zm[:], ACT.Square,
                                             accum_out=dst[:, k:k + 1])
                # front-load the 64 replicated cov matmuls into chunks 0-3
                # so the eigenvalue chain can start while rolling finishes
                if DO_COV and k < 4:
                    for i in range(16):
                        t_ = xfp[:, (k * 16 + i) * 128:(k * 16 + i + 1) * 128]
                        nc.tensor.matmul(covq[:], lhsT=t_, rhs=t_,
                                         start=(k == 0 and i == 0),
                                         stop=(k == 3 and i == 15),
                                         skip_group_check=True)
                if DO_COV and k == 4:
                    cov_post()
                if DO_COV and DO_EIG and k >= 5:
                    eig_steps(3 * (k - 5), 3 * (k - 4))

            if DO_ROLL:
                # roll quadratic sums: n = sum(zm^2) - q^2, batched [128,8]
                n20 = sml.tile([128, 8], f32, tag="n20")
                nc.vector.tensor_tensor(n20[:], num20[:], qsq20[:],
                                        op=ALU.subtract)
                n10 = sml.tile([128, 8], f32, tag="n10")
                nc.vector.tensor_tensor(n10[:], num10[:], qsq10[:],
                                        op=ALU.subtract)
                # phase locking count: n20 > thresh, masked valid
                cmp = sml.tile([128, 8], f32, tag="cmp")
                nc.vector.tensor_scalar(cmp[:], n20[:], THRESH20, None,
                                        ALU.is_gt)
                cmp2 = sml.tile([128, 8], f32, tag="cmp2")
                nc.gpsimd.tensor_mul(cmp2[:], cmp[:], v20)
                cnt = sml.tile([128, 1], f32, tag="cnt")
                nc.vector.tensor_reduce(cnt[:], cmp2[:], axis=AX.X,
                                        op=ALU.add)
                nc.vector.tensor_copy(slot(S_COUNT20), psum_scalar(cnt[:])[:])
                hv = sml.tile([128, 8], f32, tag="hv")
                nc.gpsimd.tensor_mul(hv[:], n10[:], h10)
                hs = sml.tile([128, 1], f32, tag="hs")
                nc.vector.tensor_reduce(hs[:], hv[:], axis=AX.X, op=ALU.add)
                nc.vector.tensor_copy(slot(S_HIST10), psum_scalar(hs[:])[:])
                rv = sml.tile([128, 8], f32, tag="rv")
                nc.gpsimd.tensor_mul(rv[:], n10[:], r10)
                rs = sml.tile([128, 1], f32, tag="rs")
                nc.vector.tensor_reduce(rs[:], rv[:], axis=AX.X, op=ALU.add)
                nc.vector.tensor_copy(slot(S_RECENT10), psum_scalar(rs[:])[:])

            # ---- cross-sectional std finish ----
            if DO_CS:
                cs_sq = sml.tile([128, 8], f32, tag="cs_sq")
                nc.scalar.activation(cs_sq[:], cs_s[:], ACT.Square)
                cs_var = sml.tile([128, 8], f32, tag="cs_var")
                nc.vector.scalar_tensor_tensor(
                    cs_var[:], in0=cs_sq[:], scalar=-1.0 / A, in1=cs_q[:],
                    op0=ALU.mult, op1=ALU.add)
                csstd = per.tile([128, 8], f32, tag="csstd")
                nc.scalar.activation(csstd[:], cs_var[:], ACT.Sqrt,
                                     scale=1.0 / (A - 1))
                csr = sml.tile([128, 1], f32, tag="csr")
                nc.vector.tensor_reduce(csr[:], csstd[:], axis=AX.X,
                                        op=ALU.add)
                nc.vector.tensor_copy(slot(S_CSSUM), psum_scalar(csr[:])[:])
                nc.vector.tensor_copy(slot(S_CSFIRST), csstd[0:1, 0:1])
                cslast_p = psum([1, 1], "sc")
                nc.tensor.matmul(cslast_p[:], lhsT=oh127, rhs=csstd[:, 7:8],
                                 start=True, stop=True, skip_group_check=True)
                nc.vector.tensor_copy(slot(S_CSLAST), cslast_p[:])

            # ================= write out =================
            nc.sync.dma_start(out_d[:, :], out_sb[:])

    nc.compile()
    return nc


def _prep_in_maps(inputs):
    import ml_dtypes
    bfloat16 = ml_dtypes.bfloat16
    x = np.ascontiguousarray(np.asarray(inputs["returns_sequence"],
                                        dtype=np.float32))
    xb = x.astype(bfloat16)
    m20, m10 = _build_masks()
    b0, b1 = _build_bands()

    cpack = np.zeros((128, CP_N), np.float32)
    cpack[:, CP_IDENT:CP_IDENT + 128] = np.eye(128, dtype=np.float32)
    cpack[:, CP_M20:CP_M20 + R20] = m20
    cpack[:, CP_M10:CP_M10 + R10] = m10
    w1 = np.asarray(inputs["w1"], np.float32)
    cpack[:, CP_W1A:CP_W1A + 128] = w1[0:128]
    cpack[:, CP_W1B:CP_W1B + 128] = w1[128:256]
    cpack[:, CP_B1] = np.asarray(inputs["b1"], np.float32)
    cpack[:, CP_GAM] = np.asarray(inputs["gamma"], np.float32)
    cpack[:, CP_BET] = np.asarray(inputs["beta"], np.float32)
    cpack[:, CP_W2:CP_W2 + 64] = np.asarray(inputs["w2"], np.float32)
    cpack[0:64, CP_B2] = np.asarray(inputs["b2"], np.float32)
    cpack[0:64, CP_W3:CP_W3 + 3] = np.asarray(inputs["w3"], np.float32)
    cpack[0:3, CP_B3] = np.asarray(inputs["b3"], np.float32)
    cpack[2, CP_OH2] = 1.0
    cpack[127, CP_OH127] = 1.0
    cpack[:, CP_POS] = np.asarray(inputs["positions"], np.float32)
    cpack[:, CP_XLAST] = x[-1]

    # partition-major full x: col block i is rows [i*128,(i+1)*128)
    xfull_pm = np.ascontiguousarray(
        xb.reshape(64, 128, 128).transpose(1, 0, 2).reshape(128, 64 * 128))

    in_maps = []
    for c in range(NC_N):
        rows = (c * CHUNK + np.arange(XROWS)) % T
        v20, h10, r10 = _core_masks(c)
        cp = cpack.copy()
        cp[:, CP_V20:CP_V20 + 8] = v20
        cp[:, CP_H10:CP_H10 + 8] = h10
        cp[:, CP_R10:CP_R10 + 8] = r10
        xcb = np.ascontiguousarray(xb[rows])
        xchunk_pm = np.ascontiguousarray(
            xcb.reshape(NBLK, 128, 128).transpose(1, 0, 2)
            .reshape(128, XROWS))
        bpack = np.zeros((128, BP_N), bfloat16)
        bpack[:, BP_B0:BP_B0 + 256] = b0.astype(bfloat16)
        bpack[:, BP_B1:BP_B1 + 256] = b1.astype(bfloat16)
        bpack[:, BP_XT:BP_XT + XROWS] = xcb.T
        in_maps.append({
            "x_full_pm": xfull_pm,
            "xchunk_pm": xchunk_pm,
            "cpack": cp,
            "bpack": bpack,
        })
    return in_maps


def _combine(per_core):
    count20 = sum(float(per_core[c][0, S_COUNT20]) for c in range(NC_N))
    hist_raw = sum(float(per_core[c][0, S_HIST10]) for c in range(NC_N))
    rec_raw = sum(float(per_core[c][0, S_RECENT10]) for c in range(NC_N))
    cs_sum = sum(float(per_core[c][0, S_CSSUM]) for c in range(NC_N))
    ssq_sum = sum(float(per_core[c][0, S_SSQ]) for c in range(NC_N))
    cs_first = float(per_core[0][0, S_CSFIRST])
    cs_last = float(per_core[NC_N - 1][0, S_CSLAST])
    r0 = per_core[0][0]
    sum_corr = float(r0[S_SUMCORR])
    sum_abs = float(r0[S_SUMABS])
    trace_c = float(r0[S_TRACE])
    pa_sum = float(r0[S_PASUM])
    pa_max = float(r0[S_PAMAX])
    severity = float(r0[S_SEV])
    T9 = float(r0[S_T9])

    phase_locking = count20 / N20
    nh = N10 - 5
    hist = (hist_raw - nh * A) * INV_OD / nh
    recent = (rec_raw - 5 * A) * INV_OD / 5.0
    surge = 0.0
    if hist > 0:
        surge = min(max((recent - hist) / hist, 0.0), 1.0)
    avg_disp = cs_sum / T
    trend = -(cs_last - cs_first) / (T - 1)
    herding_index = min(max(trend / (avg_disp + 1e-6) + 0.5, 0.0), 1.0)
    avg_corr = (sum_corr - trace_c) / (A * (A - 1))
    # T9 = trace(corr^512) / EIG_C^8 on device; lam ~ trace(corr^512)^(1/512)
    lam = np.exp((8.0 * np.log(EIG_C) + np.log(T9)) / 512.0)
    sync_risk = min(1.0, (lam / A) * avg_corr)
    return_div = 1.0 - sum_abs / (A * A)
    pos_div = 1.0 - pa_max / pa_sum
    div_loss = 1.0 - np.sqrt(return_div * pos_div)
    avg_conc = (A * A / 2.0 + ssq_sum / (2.0 * T) - A) / (A * (A - 1))
    phase_coupling = min(max((avg_conc - 0.5) * 2.0, 0.0), 1.0)
    collective = (herding_index + sync_risk + div_loss) / 3.0
    return np.array([herding_index, severity, sync_risk, phase_locking,
                     div_loss, surge, phase_coupling, collective],
                    dtype=np.float32)


def _ensure_ntff_hook():
    """Install the axon NTFF profile hook if the image lacks antenv.axon_hooks."""
    import sys
    import types
    try:
        import antenv.axon_hooks  # noqa: F401
        return True
    except ImportError:
        pass
    try:
        import antenv
        from trn_agent_boot.trn_boot import _ntff_profile_via_ctypes
        mod = types.ModuleType("antenv.axon_hooks")
        state = {}
        mod.set_axon_ntff_profile_hook = lambda h: state.update(h=h)
        mod.get_axon_ntff_profile_hook = lambda: state.get("h")
        sys.modules["antenv.axon_hooks"] = mod
        antenv.axon_hooks = mod
        hook = _ntff_profile_via_ctypes("/opt/axon/libaxon_pjrt.so")
        mod.set_axon_ntff_profile_hook(hook)
        return hook is not None
    except Exception:
        return False


def _run(inputs, trace=False):
    from concourse.bass_utils import run_bass_kernel_spmd
    if trace:
        trace = _ensure_ntff_hook()
    if "nc" not in _PLAN:
        _PLAN["nc"] = _build_program()
    nc = _PLAN["nc"]
    in_maps = _prep_in_maps(inputs)
    res = run_bass_kernel_spmd(nc, in_maps, core_ids=list(range(NC_N)),
                               trace=trace)
    per_core = [res.results[c]["out_vec"] for c in range(NC_N)]
    return _combine(per_core), res


def kernel(**inputs) -> np.ndarray:
    out, _ = _run(inputs, trace=False)
    return out


# revision 25
# speedup vs baseline: 1.0393x; 1.0216x over previous
"""Trainium2 Bass kernel for nn_EmergentRiskMetrics.

Contract: kernel(**inputs) takes the FULL unsharded inputs (as produced by
setup_inputs()) and returns the FULL output (shape [8], float32).

Sharding: data-parallel over the time axis. Each of the 8 cores owns 1024
contiguous window starts (plus a 128-row halo) for the two rolling-window
correlation scans; the sign-concordance partial sum and cross-sectional
stds are computed on the owning core and combined as scalars on the host.
The full-T covariance (needed on-device for the eigenvalue iteration) is
replicated: every core re-computes X^T X from bf16 tiles of the full
sequence (~1 us of 128^3 bf16 matmuls + ~2 MB of DMA, overlapped with the
rolling phase). An AllReduce-based variant was measured at ~66 us of pure
collective latency for 64 KB on this runtime — replication is far cheaper.

DMA-trigger serialization dominates small-tensor staging, so the host
packs every fp32 constant (masks, identity, MLP weights, positions, ...)
into ONE [128,1024] tensor, and the bf16 bands + pre-transposed chunk
into ONE [128,1664] tensor; x_full lands via 8 big strided DMAs split
across the two HWDGE queues (sync + scalar).

Windowed sums are banded-matrix matmuls on the tensor engine (bands
pre-scaled by 1/sqrt(w) so the mean-correction q^2 term folds into the
V-mask), u = 1/std via reciprocal_approx_fast + Sqrt, and the whole
rolling path runs in bf16 (validated: all rolling-derived outputs have
orders-of-magnitude margin against bf16 noise; d2 >= 0 holds exactly
because S and P derive from the same bf16 x). V*mask+reduce is fused via
tensor_tensor_reduce.

Top eigenvalue: corr is squared 9 times in bf16 (fp32 PSUM accumulate);
traces at step 6 (normalization) and step 9 give lam = (T9*T6^8)^(1/512)
on the host.

Device outputs are per-core partial scalars; the host only gathers them
(sums partial sums, applies the final scalar clips/divides) to assemble
the 8 outputs.
"""

import numpy as np

T = 8192
A = 128
W20 = 20
W10 = 10
NC_N = 8
CHUNK = 1024            # window starts per core
XROWS = 1152            # rows of per-core chunk (9 x 128, incl. halo)
NBLK = XROWS // 128     # 9
R20 = 128 + W20 - 1     # 147
R10 = 128 + W10 - 1     # 137
N20 = T - W20           # 8172 rolling-20 windows
N10 = T - W10           # 8182 rolling-10 windows
OUT_SLOTS = 24
INV_OD = 1.0 / (A * (A - 1))
# rolling20 > 0.7 in corr units == raw quadratic sum > this
THRESH20 = 0.7 * (A * (A - 1)) + A
# static normalization for the eigenvalue squaring chain (~trace(corr^64));
# only needs to be within ~e+-80 of the true value for fp32/bf16 range
EIG_C = 4.0e6

S_COUNT20, S_HIST10, S_RECENT10, S_CSSUM, S_CSFIRST, S_CSLAST, \
    S_SUMCORR, S_SUMABS, S_TRACE, S_PASUM, S_PAMAX, S_SEV, S_SSQ, \
    S_T6, S_T9 = range(15)

# packed fp32 constant tensor column layout
CP_IDENT = 0
CP_M20 = 128                 # 275 cols
CP_M10 = CP_M20 + R20 + 128  # 403, 265 cols
CP_V20 = CP_M10 + R10 + 128  # 668
CP_H10 = CP_V20 + 8
CP_R10 = CP_H10 + 8
CP_W1A = CP_R10 + 8          # 692
CP_W1B = CP_W1A + 128        # 820
CP_B1 = CP_W1B + 128         # 948
CP_GAM = CP_B1 + 1
CP_BET = CP_GAM + 1
CP_W2 = CP_BET + 1           # 951, 64 cols
CP_B2 = CP_W2 + 64           # 1015
CP_W3 = CP_B2 + 1            # 1016, 3 cols
CP_B3 = CP_W3 + 3            # 1019
CP_OH2 = CP_B3 + 1
CP_OH127 = CP_OH2 + 1
CP_POS = CP_OH127 + 1
CP_XLAST = CP_POS + 1
CP_N = CP_XLAST + 1          # 1024

BP_B0 = 0
BP_B1 = 256
BP_XT = 512
BP_N = BP_XT + XROWS         # 1664

_PLAN = {}


def _build_masks():
    # 0/1 window-membership bands (the q^2 term is handled separately)
    m20 = np.zeros((128, R20), np.float32)
    m10 = np.zeros((128, R10), np.float32)
    for j in range(128):
        m20[j, j:j + W20] = 1.0
        m10[j, j:j + W10] = 1.0
    return m20, m10


def _build_bands():
    # bands0/1 [128 t, 256]: cols 0:128 window-20 (scaled 1/sqrt20),
    # cols 128:256 window-10 (scaled 1/sqrt10). S' = B0^T x_k + B1^T x_{k+1}
    b0 = np.zeros((128, 256), np.float32)
    b1 = np.zeros((128, 256), np.float32)
    s20 = 1.0 / np.sqrt(W20)
    s10 = 1.0 / np.sqrt(W10)
    for j in range(128):
        b0[j:min(128, j + W20), j] = s20
        if j + W20 > 128:
            b1[0:j + W20 - 128, j] = s20
        b0[j:min(128, j + W10), 128 + j] = s10
        if j + W10 > 128:
            b1[0:j + W10 - 128, 128 + j] = s10
    return b0, b1


def _core_masks(c):
    g = c * CHUNK + np.arange(CHUNK)
    valid20 = (g < N20).astype(np.float32)
    hist10 = (g < N10 - 5).astype(np.float32)
    recent10 = ((g >= N10 - 5) & (g < N10)).astype(np.float32)
    # device layout [128 partitions (j in chunk), 8 chunk-columns]
    return (np.ascontiguousarray(valid20.reshape(8, 128).T),
            np.ascontiguousarray(hist10.reshape(8, 128).T),
            np.ascontiguousarray(recent10.reshape(8, 128).T))


def _build_program():
    import os
    import concourse.bacc as bacc
    import concourse.tile as tile
    from concourse import mybir

    kbits = int(os.environ.get("KBITS", "63"))
    bigdma = int(os.environ.get("BIGDMA", "1"))
    # tensor_tensor_reduce hard-crashes the exec unit on this runtime
    use_ttr = int(os.environ.get("TTR", "0"))
    DO_ROLL = kbits & 1
    DO_CS = kbits & 2
    DO_COV = kbits & 4
    DO_EIG = kbits & 8
    DO_POS = kbits & 16
    DO_MLP = kbits & 32

    f32 = mybir.dt.float32
    bf16 = mybir.dt.bfloat16
    ALU = mybir.AluOpType
    ACT = mybir.ActivationFunctionType
    AX = mybir.AxisListType

    nc = bacc.Bacc("TRN2", target_bir_lowering=False, debug=False,
                   num_devices=NC_N)

    def din(name, shape, dt=f32):
        return nc.dram_tensor(name, shape, dt, kind="ExternalInput").ap()

    # partition-major layouts (host pre-permuted): col block i of x_full_pm
    # is x[i*128:(i+1)*128, :] with time-on-partitions — plain contiguous
    # DMAs with one descriptor per partition.
    x_full_pm = din("x_full_pm", [128, 64 * 128], bf16)
    xchunk_pm = din("xchunk_pm", [128, XROWS], bf16)
    cpack_in = din("cpack", [128, CP_N])
    bpack_in = din("bpack", [128, BP_N], bf16)
    out_d = nc.dram_tensor("out_vec", [1, OUT_SLOTS], f32,
                           kind="ExternalOutput").ap()

    with tile.TileContext(nc) as tc:
        with tc.tile_pool(name="const", bufs=1) as cst, \
             tc.tile_pool(name="persist", bufs=1) as per, \
             tc.tile_pool(name="sgs", bufs=3) as sgs, \
             tc.tile_pool(name="wrk", bufs=3) as wrk, \
             tc.tile_pool(name="small", bufs=6) as sml, \
             tc.tile_pool(name="ps", bufs=1, space="PSUM") as ps:

            psum_bufs = {"covq": 1, "band": 2, "zp": 2, "big": 1, "sc": 2}

            def psum(shape, tag):
                return ps.tile(shape, f32, tag=tag, name=tag,
                               bufs=psum_bufs[tag])

            # ---- packed loads: bpack/xchunk on sync, cpack on scalar ----
            bpk = cst.tile([128, BP_N], bf16, tag="bpk")
            nc.sync.dma_start(bpk[:], bpack_in[:, :])
            xck = per.tile([128, XROWS], bf16, tag="xck")
            nc.sync.dma_start(xck[:], xchunk_pm[:, :])
            cpk = cst.tile([128, CP_N], f32, tag="cpk")
            nc.scalar.dma_start(cpk[:], cpack_in[:, :])

            b0b = bpk[:, BP_B0:BP_B0 + 256]
            b1b = bpk[:, BP_B1:BP_B1 + 256]
            xTb = bpk[:, BP_XT:BP_XT + XROWS]
            xcbs = [xck[:, j * 128:(j + 1) * 128] for j in range(NBLK)]

            ident = cpk[:, CP_IDENT:CP_IDENT + 128]
            # dedicated mask tiles (vector TENSOR_TENSOR against a slice of
            # the wide packed tile crashed the exec unit)
            m20t = cst.tile([128, R20], f32, tag="m20t")
            nc.gpsimd.tensor_copy(m20t[:], cpk[:, CP_M20:CP_M20 + R20])
            m10t = cst.tile([128, R10], f32, tag="m10t")
            nc.gpsimd.tensor_copy(m10t[:], cpk[:, CP_M10:CP_M10 + R10])
            m20 = m20t[:]
            m10 = m10t[:]
            v20 = cpk[:, CP_V20:CP_V20 + 8]
            h10 = cpk[:, CP_H10:CP_H10 + 8]
            r10 = cpk[:, CP_R10:CP_R10 + 8]
            w1a = cpk[:, CP_W1A:CP_W1A + 128]
            w1b = cpk[:, CP_W1B:CP_W1B + 128]
            b1 = cpk[:, CP_B1:CP_B1 + 1]
            gam = cpk[:, CP_GAM:CP_GAM + 1]
            bet = cpk[:, CP_BET:CP_BET + 1]
            w2 = cpk[:, CP_W2:CP_W2 + 64]
            b2 = cpk[0:64, CP_B2:CP_B2 + 1]
            w3 = cpk[0:64, CP_W3:CP_W3 + 3]
            b3 = cpk[0:3, CP_B3:CP_B3 + 1]
            oh2 = cpk[0:3, CP_OH2:CP_OH2 + 1]
            oh127 = cpk[:, CP_OH127:CP_OH127 + 1]
            pos_sb = cpk[:, CP_POS:CP_POS + 1]
            xl = cpk[:, CP_XLAST:CP_XLAST + 1]

            ones = cst.tile([128, 1], f32, tag="ones")
            nc.vector.memset(ones[:], 1.0)
            onesb = cst.tile([128, 1], bf16, tag="onesb")
            nc.vector.memset(onesb[:], 1.0)
            ones_row = cst.tile([1, 128], f32, tag="ones_row")
            nc.vector.memset(ones_row[:], 1.0)

            out_sb = per.tile([1, OUT_SLOTS], f32, tag="out_sb")
            nc.vector.memset(out_sb[:], 0.0)

            def slot(i):
                return out_sb[:, i:i + 1]

            def psum_scalar(vec_sb, p=128):
                o = psum([1, 1], "sc")
                lhs = ones[0:p, :] if p != 128 else ones[:]
                nc.tensor.matmul(o[:], lhsT=lhs, rhs=vec_sb,
                                 start=True, stop=True, skip_group_check=True)
                return o

            # ---- full x for replicated cov: 2 halves on the 2 HWDGE queues
            xfp = per.tile([128, 64 * 128], bf16, tag="xfp")
            if DO_COV:
                # 4 quarters alternating queues; cov chunk k consumes
                # quarter k, so the earliest-landing quarters go first
                for i in range(4):
                    eng = nc.sync if i % 2 == 0 else nc.scalar
                    eng.dma_start(xfp[:, i * 2048:(i + 1) * 2048],
                                  x_full_pm[:, i * 2048:(i + 1) * 2048])

            # ---- per-tile squares (gpsimd; reads SBUF only) ----
            xsqbs = []
            for j in range(NBLK):
                xsqb = per.tile([128, 128], bf16, tag="xsqb%d" % j)
                nc.gpsimd.tensor_mul(xsqb[:], xcbs[j], xcbs[j])
                xsqbs.append(xsqb)

            # ---- sharded sign concordance ----
            mq = psum([128, 128], "big")
            for i in range(8):
                sg = sgs.tile([128, 128], bf16, tag="sg")
                nc.scalar.activation(sg[:], xcbs[i], ACT.Sign)
                nc.tensor.matmul(mq[:], lhsT=sg[:], rhs=sg[:],
                                 start=(i == 0), stop=(i == 7),
                                 skip_group_check=True)
            mr = sml.tile([128, 1], f32, tag="mr")
            nc.vector.tensor_reduce(mr[:], mq[:], axis=AX.X, op=ALU.add)
            nc.vector.tensor_copy(slot(S_SSQ), psum_scalar(mr[:])[:])

            # ---- cross-sectional sums (independent; fills startup) ----
            if DO_CS:
                cs_s = per.tile([128, 8], f32, tag="cs_s")
                cs_q = per.tile([128, 8], f32, tag="cs_q")
                for b in range(8):
                    nc.vector.tensor_reduce(cs_s[:, b:b + 1], xcbs[b],
                                            axis=AX.X, op=ALU.add)
                    nc.vector.tensor_reduce(cs_q[:, b:b + 1], xsqbs[b][:],
                                            axis=AX.X, op=ALU.add)

            # ================= position diversity =================
            if DO_POS:
                pa = per.tile([128, 1], f32, tag="pa")
                nc.scalar.activation(pa[:], pos_sb, ACT.Abs)
                nc.vector.tensor_copy(slot(S_PASUM), psum_scalar(pa[:])[:])
                paT_p = psum([1, 128], "sc")
                nc.tensor.transpose(paT_p[:], pa[:], ident)
                paT = sml.tile([1, 128], f32, tag="paT")
                nc.vector.tensor_copy(paT[:], paT_p[:])
                nc.vector.tensor_reduce(slot(S_PAMAX), paT[:], axis=AX.X,
                                        op=ALU.max)

            # ================= herding MLP =================
            if DO_MLP:
                h1p = psum([128, 1], "sc")
                nc.tensor.matmul(h1p[:], lhsT=w1a, rhs=xl, start=True,
                                 stop=False, skip_group_check=True)
                nc.tensor.matmul(h1p[:], lhsT=w1b, rhs=pos_sb,
                                 start=False, stop=True,
                                 skip_group_check=True)
                h1 = sml.tile([128, 1], f32, tag="h1")
                nc.scalar.activation(h1[:], h1p[:], ACT.Relu, bias=b1)
                gk = sml.tile([128, 1], f32, tag="gk")
                nc.vector.tensor_scalar(gk[:], gam,
                                        float(1.0 / np.sqrt(1.0 + 1e-5)),
                                        None, ALU.mult)
                h1b = sml.tile([128, 1], f32, tag="h1b")
                nc.vector.tensor_scalar(h1b[:], h1[:], gk[:], bet,
                                        ALU.mult, ALU.add)
                h2p = psum([64, 1], "sc")
                nc.tensor.matmul(h2p[:], lhsT=w2, rhs=h1b[:], start=True,
                                 stop=True, skip_group_check=True)
                h2 = sml.tile([64, 1], f32, tag="h2")
                nc.scalar.activation(h2[:], h2p[:], ACT.Relu, bias=b2)
                lg = psum([3, 1], "sc")
                nc.tensor.matmul(lg[:], lhsT=w3, rhs=h2[:], start=True,
                                 stop=True, skip_group_check=True)
                exps = sml.tile([3, 1], f32, tag="exps")
                nc.scalar.activation(exps[:], lg[:], ACT.Exp, bias=b3)
                esum = psum_scalar(exps[:], p=3)
                esum_sb = sml.tile([1, 1], f32, tag="esum_sb")
                nc.vector.tensor_copy(esum_sb[:], esum[:])
                erec = sml.tile([1, 1], f32, tag="erec")
                nc.vector.reciprocal(erec[:], esum_sb[:])
                e2p = psum([1, 1], "sc")
                nc.tensor.matmul(e2p[:], lhsT=oh2, rhs=exps[:], start=True,
                                 stop=True, skip_group_check=True)
                e2_sb = sml.tile([1, 1], f32, tag="e2_sb")
                nc.vector.tensor_copy(e2_sb[:], e2p[:])
                nc.vector.tensor_mul(slot(S_SEV), e2_sb[:], erec[:])

            # ---- cov post + eig emitted as closures, woven into the loop ----
            eig_state = {}

            def cov_post():
                cov = per.tile([128, 128], f32, tag="cov")
                nc.scalar.activation(cov[:], covq[:], ACT.Copy)
                dscr = wrk.tile([128, 128], f32, tag="dscr")
                nc.vector.tensor_mul(dscr[:], cov[:], ident)
                diag = per.tile([128, 1], f32, tag="diag")
                nc.vector.tensor_reduce(diag[:], dscr[:], axis=AX.X,
                                        op=ALU.add)
                dstd = per.tile([128, 1], f32, tag="dstd")
                nc.scalar.activation(dstd[:], diag[:], ACT.Sqrt)
                ucol = per.tile([128, 1], f32, tag="ucol")
                nc.vector.reciprocal(ucol[:], dstd[:])
                u2 = sml.tile([128, 1], f32, tag="u2")
                nc.vector.tensor_mul(u2[:], ucol[:], ucol[:])
                du2 = sml.tile([128, 1], f32, tag="du2")
                nc.vector.tensor_mul(du2[:], u2[:], diag[:])
                nc.vector.tensor_copy(slot(S_TRACE), psum_scalar(du2[:])[:])

                uT_p = psum([1, 128], "sc")
                nc.tensor.transpose(uT_p[:], ucol[:], ident)
                uT = per.tile([1, 128], f32, tag="uT")
                nc.vector.tensor_copy(uT[:], uT_p[:])

                def quad_form(mat_sb, out_slot):
                    qr = psum([1, 128], "sc")
                    nc.tensor.matmul(qr[:], lhsT=ucol[:], rhs=mat_sb,
                                     start=True, stop=True,
                                     skip_group_check=True)
                    qscr = sml.tile([1, 128], f32, tag="qscr")
                    nc.vector.tensor_mul(qscr[:], qr[:], uT[:])
                    qacc = sml.tile([1, 1], f32, tag="qacc")
                    nc.vector.tensor_reduce(qacc[:], qscr[:], axis=AX.X,
                                            op=ALU.add)
                    nc.vector.tensor_copy(out_slot, qacc[:])

                quad_form(cov[:], slot(S_SUMCORR))
                acov = per.tile([128, 128], f32, tag="acov")
                nc.scalar.activation(acov[:], cov[:], ACT.Abs)
                quad_form(acov[:], slot(S_SUMABS))

                # corr = diag(u) cov diag(u) -> bf16
                brow = per.tile([128, 128], f32, tag="brow")
                nc.vector.tensor_scalar(brow[:], cov[:], ucol[:], None,
                                        ALU.mult)
                bt_p = psum([128, 128], "big")
                nc.tensor.transpose(bt_p[:], brow[:], ident)
                corr = per.tile([128, 128], bf16, tag="corr")
                nc.scalar.activation(corr[:], bt_p[:], ACT.Copy,
                                     scale=ucol[:])
                eig_state["M"] = corr

            def eig_steps(lo, hi):
                # squaring steps lo..hi-1; static 1/EIG_C normalization at
                # step 5; trace of corr^512 at step 8
                M = eig_state["M"]
                for kk in range(lo, hi):
                    p = psum([128, 128], "big")
                    nc.tensor.matmul(p[:], lhsT=M[:], rhs=M[:],
                                     start=True, stop=True,
                                     skip_group_check=True)
                    Mn = wrk.tile([128, 128], bf16, tag="Mn")
                    if kk == 8:
                        escr = wrk.tile([128, 128], f32, tag="escr")
                        nc.vector.tensor_mul(escr[:], p[:], ident)
                        edg = sml.tile([128, 1], f32, tag="edg")
                        nc.vector.tensor_reduce(edg[:], escr[:], axis=AX.X,
                                                op=ALU.add)
                        trp = psum_scalar(edg[:])
                        nc.vector.tensor_copy(slot(S_T9), trp[:])
                        break
                    nc.scalar.activation(Mn[:], p[:], ACT.Copy,
                                         scale=(1.0 / EIG_C if kk == 5
                                                else 1.0))
                    M = Mn
                eig_state["M"] = M

            # ====== rolling windows + cov + eig chain, interleaved ======
            covq = psum([128, 128], "covq")
            num20 = per.tile([128, 8], f32, tag="num20")
            num10 = per.tile([128, 8], f32, tag="num10")
            qsq20 = per.tile([128, 8], f32, tag="qsq20")
            qsq10 = per.tile([128, 8], f32, tag="qsq10")
            for k in range(8):
                if DO_ROLL:
                    sp_ = psum([128, 256], "band")
                    nc.tensor.matmul(sp_[:], lhsT=xcbs[k], rhs=b0b,
                                     start=True, stop=False,
                                     skip_group_check=True)
                    nc.tensor.matmul(sp_[:], lhsT=xcbs[k + 1], rhs=b1b,
                                     start=False, stop=True,
                                     skip_group_check=True)
                    pp = psum([128, 256], "band")
                    nc.tensor.matmul(pp[:], lhsT=xsqbs[k][:], rhs=b0b,
                                     start=True, stop=False,
                                     skip_group_check=True)
                    nc.tensor.matmul(pp[:], lhsT=xsqbs[k + 1][:], rhs=b1b,
                                     start=False, stop=True,
                                     skip_group_check=True)
                    # d2 = P - S^2/w = pp*sqrt(w) - S'^2 (>=0 exactly)
                    sq = wrk.tile([128, 256], f32, tag="sq")
                    nc.scalar.activation(sq[:], sp_[:], ACT.Square)
                    d2 = wrk.tile([128, 256], f32, tag="d2")
                    nc.vector.scalar_tensor_tensor(
                        d2[:, 0:128], in0=pp[:, 0:128],
                        scalar=float(np.sqrt(W20)),
                        in1=sq[:, 0:128], op0=ALU.mult, op1=ALU.subtract)
                    nc.vector.scalar_tensor_tensor(
                        d2[:, 128:256], in0=pp[:, 128:256],
                        scalar=float(np.sqrt(W10)),
                        in1=sq[:, 128:256], op0=ALU.mult, op1=ALU.subtract)
                    rd2 = wrk.tile([128, 256], f32, tag="rd2")
                    nc.vector.reciprocal_approx_fast(rd2[:], d2[:])
                    ub = wrk.tile([128, 256], bf16, tag="ub")
                    nc.scalar.activation(ub[:], rd2[:], ACT.Sqrt)

                    # q_j = sum_a u*S' via elementwise mul + ones-matmul
                    us = wrk.tile([128, 256], bf16, tag="us")
                    nc.vector.tensor_mul(us[:], ub[:], sp_[:])
                    for (wi, R, msk) in ((0, R20, m20), (1, R10, m10)):
                        qp = psum([128, 1], "sc")
                        nc.tensor.matmul(
                            qp[:], lhsT=us[:, wi * 128:(wi + 1) * 128],
                            rhs=onesb[:], start=True, stop=True,
                            skip_group_check=True)
                        qdst = qsq20 if wi == 0 else qsq10
                        nc.scalar.activation(qdst[:, k:k + 1], qp[:],
                                             ACT.Square)
                        zp = psum([128, R], "zp")
                        nc.tensor.matmul(
                            zp[:], lhsT=ub[:, wi * 128:(wi + 1) * 128],
                            rhs=xTb[:, k * 128:k * 128 + R],
                            start=True, stop=True, skip_group_check=True)
                        # mask to the window band, then square+row-reduce in
                        # one scalar-engine activation (accum_out)
                        zm = wrk.tile([128, R], bf16, tag="zm%d" % wi)
                        nc.vector.tensor_mul(zm[:], zp[:], msk)
                        V = wrk.tile([128, R], bf16, tag="V%d" % wi)
                        dst = num20 if wi == 0 else num10
                        nc.scalar.activation(V[:], zm[:], ACT.Square,
                                             accum_out=dst[:, k:k + 1])
                # front-load the 64 replicated cov matmuls into chunks 0-4
                # so the eigenvalue chain can start while rolling finishes
                if DO_COV and k < 5:
                    lo = [0, 13, 26, 39, 52][k]
                    hi = [13, 26, 39, 52, 64][k]
                    for i in range(lo, hi):
                        t_ = xfp[:, i * 128:(i + 1) * 128]
                        nc.tensor.matmul(covq[:], lhsT=t_, rhs=t_,
                                         start=(i == 0), stop=(i == 63),
                                         skip_group_check=True)
                if DO_COV and k == 5:
                    cov_post()
                if DO_COV and DO_EIG and k >= 6:
                    eig_steps(4 * (k - 6), 4 * (k - 5))
            if DO_COV and DO_EIG:
                eig_steps(8, 9)

            if DO_ROLL:
                # roll quadratic sums: n = sum(zm^2) - q^2, batched [128,8]
                n20 = sml.tile([128, 8], f32, tag="n20")
                nc.vector.tensor_tensor(n20[:], num20[:], qsq20[:],
                                        op=ALU.subtract)
                n10 = sml.tile([128, 8], f32, tag="n10")
                nc.vector.tensor_tensor(n10[:], num10[:], qsq10[:],
                                        op=ALU.subtract)
                # phase locking count: n20 > thresh, masked valid
                cmp = sml.tile([128, 8], f32, tag="cmp")
                nc.vector.tensor_scalar(cmp[:], n20[:], THRESH20, None,
                                        ALU.is_gt)
                cmp2 = sml.tile([128, 8], f32, tag="cmp2")
                nc.gpsimd.tensor_mul(cmp2[:], cmp[:], v20)
                cnt = sml.tile([128, 1], f32, tag="cnt")
                nc.vector.tensor_reduce(cnt[:], cmp2[:], axis=AX.X,
                                        op=ALU.add)
                nc.vector.tensor_copy(slot(S_COUNT20), psum_scalar(cnt[:])[:])
                hv = sml.tile([128, 8], f32, tag="hv")
                nc.gpsimd.tensor_mul(hv[:], n10[:], h10)
                hs = sml.tile([128, 1], f32, tag="hs")
                nc.vector.tensor_reduce(hs[:], hv[:], axis=AX.X, op=ALU.add)
                nc.vector.tensor_copy(slot(S_HIST10), psum_scalar(hs[:])[:])
                rv = sml.tile([128, 8], f32, tag="rv")
                nc.gpsimd.tensor_mul(rv[:], n10[:], r10)
                rs = sml.tile([128, 1], f32, tag="rs")
                nc.vector.tensor_reduce(rs[:], rv[:], axis=AX.X, op=ALU.add)
                nc.vector.tensor_copy(slot(S_RECENT10), psum_scalar(rs[:])[:])

            # ---- cross-sectional std finish ----
            if DO_CS:
                cs_sq = sml.tile([128, 8], f32, tag="cs_sq")
                nc.scalar.activation(cs_sq[:], cs_s[:], ACT.Square)
                cs_var = sml.tile([128, 8], f32, tag="cs_var")
                nc.vector.scalar_tensor_tensor(
                    cs_var[:], in0=cs_sq[:], scalar=-1.0 / A, in1=cs_q[:],
                    op0=ALU.mult, op1=ALU.add)
                csstd = per.tile([128, 8], f32, tag="csstd")
                nc.scalar.activation(csstd[:], cs_var[:], ACT.Sqrt,
                                     scale=1.0 / (A - 1))
                csr = sml.tile([128, 1], f32, tag="csr")
                nc.vector.tensor_reduce(csr[:], csstd[:], axis=AX.X,
                                        op=ALU.add)
                nc.vector.tensor_copy(slot(S_CSSUM), psum_scalar(csr[:])[:])
                nc.vector.tensor_copy(slot(S_CSFIRST), csstd[0:1, 0:1])
                cslast_p = psum([1, 1], "sc")
                nc.tensor.matmul(cslast_p[:], lhsT=oh127, rhs=csstd[:, 7:8],
                                 start=True, stop=True, skip_group_check=True)
                nc.vector.tensor_copy(slot(S_CSLAST), cslast_p[:])

            # ================= write out =================
            nc.sync.dma_start(out_d[:, :], out_sb[:])

    nc.compile()
    return nc


def _prep_in_maps(inputs):
    import ml_dtypes
    bfloat16 = ml_dtypes.bfloat16
    x = np.ascontiguousarray(np.asarray(inputs["returns_sequence"],
                                        dtype=np.float32))
    xb = x.astype(bfloat16)
    m20, m10 = _build_masks()
    b0, b1 = _build_bands()

    cpack = np.zeros((128, CP_N), np.float32)
    cpack[:, CP_IDENT:CP_IDENT + 128] = np.eye(128, dtype=np.float32)
    cpack[:, CP_M20:CP_M20 + R20] = m20
    cpack[:, CP_M10:CP_M10 + R10] = m10
    w1 = np.asarray(inputs["w1"], np.float32)
    cpack[:, CP_W1A:CP_W1A + 128] = w1[0:128]
    cpack[:, CP_W1B:CP_W1B + 128] = w1[128:256]
    cpack[:, CP_B1] = np.asarray(inputs["b1"], np.float32)
    cpack[:, CP_GAM] = np.asarray(inputs["gamma"], np.float32)
    cpack[:, CP_BET] = np.asarray(inputs["beta"], np.float32)
    cpack[:, CP_W2:CP_W2 + 64] = np.asarray(inputs["w2"], np.float32)
    cpack[0:64, CP_B2] = np.asarray(inputs["b2"], np.float32)
    cpack[0:64, CP_W3:CP_W3 + 3] = np.asarray(inputs["w3"], np.float32)
    cpack[0:3, CP_B3] = np.asarray(inputs["b3"], np.float32)
    cpack[2, CP_OH2] = 1.0
    cpack[127, CP_OH127] = 1.0
    cpack[:, CP_POS] = np.asarray(inputs["positions"], np.float32)
    cpack[:, CP_XLAST] = x[-1]

    # partition-major full x: col block i is rows [i*128,(i+1)*128)
    xfull_pm = np.ascontiguousarray(
        xb.reshape(64, 128, 128).transpose(1, 0, 2).reshape(128, 64 * 128))

    in_maps = []
    for c in range(NC_N):
        rows = (c * CHUNK + np.arange(XROWS)) % T
        v20, h10, r10 = _core_masks(c)
        cp = cpack.copy()
        cp[:, CP_V20:CP_V20 + 8] = v20
        cp[:, CP_H10:CP_H10 + 8] = h10
        cp[:, CP_R10:CP_R10 + 8] = r10
        xcb = np.ascontiguousarray(xb[rows])
        xchunk_pm = np.ascontiguousarray(
            xcb.reshape(NBLK, 128, 128).transpose(1, 0, 2)
            .reshape(128, XROWS))
        bpack = np.zeros((128, BP_N), bfloat16)
        bpack[:, BP_B0:BP_B0 + 256] = b0.astype(bfloat16)
        bpack[:, BP_B1:BP_B1 + 256] = b1.astype(bfloat16)
        bpack[:, BP_XT:BP_XT + XROWS] = xcb.T
        in_maps.append({
            "x_full_pm": xfull_pm,
            "xchunk_pm": xchunk_pm,
            "cpack": cp,
            "bpack": bpack,
        })
    return in_maps


def _combine(per_core):
    count20 = sum(float(per_core[c][0, S_COUNT20]) for c in range(NC_N))
    hist_raw = sum(float(per_core[c][0, S_HIST10]) for c in range(NC_N))
    rec_raw = sum(float(per_core[c][0, S_RECENT10]) for c in range(NC_N))
    cs_sum = sum(float(per_core[c][0, S_CSSUM]) for c in range(NC_N))
    ssq_sum = sum(float(per_core[c][0, S_SSQ]) for c in range(NC_N))
    cs_first = float(per_core[0][0, S_CSFIRST])
    cs_last = float(per_core[NC_N - 1][0, S_CSLAST])
    r0 = per_core[0][0]
    sum_corr = float(r0[S_SUMCORR])
    sum_abs = float(r0[S_SUMABS])
    trace_c = float(r0[S_TRACE])
    pa_sum = float(r0[S_PASUM])
    pa_max = float(r0[S_PAMAX])
    severity = float(r0[S_SEV])
    T9 = float(r0[S_T9])

    phase_locking = count20 / N20
    nh = N10 - 5
    hist = (hist_raw - nh * A) * INV_OD / nh
    recent = (rec_raw - 5 * A) * INV_OD / 5.0
    surge = 0.0
    if hist > 0:
        surge = min(max((recent - hist) / hist, 0.0), 1.0)
    avg_disp = cs_sum / T
    trend = -(cs_last - cs_first) / (T - 1)
    herding_index = min(max(trend / (avg_disp + 1e-6) + 0.5, 0.0), 1.0)
    avg_corr = (sum_corr - trace_c) / (A * (A - 1))
    # T9 = trace(corr^512) / EIG_C^8 on device; lam ~ trace(corr^512)^(1/512)
    lam = np.exp((8.0 * np.log(EIG_C) + np.log(T9)) / 512.0)
    sync_risk = min(1.0, (lam / A) * avg_corr)
    return_div = 1.0 - sum_abs / (A * A)
    pos_div = 1.0 - pa_max / pa_sum
    div_loss = 1.0 - np.sqrt(return_div * pos_div)
    avg_conc = (A * A / 2.0 + ssq_sum / (2.0 * T) - A) / (A * (A - 1))
    phase_coupling = min(max((avg_conc - 0.5) * 2.0, 0.0), 1.0)
    collective = (herding_index + sync_risk + div_loss) / 3.0
    return np.array([herding_index, severity, sync_risk, phase_locking,
                     div_loss, surge, phase_coupling, collective],
                    dtype=np.float32)


def _ensure_ntff_hook():
    """Install the axon NTFF profile hook if the image lacks antenv.axon_hooks."""
    import sys
    import types
    try:
        import antenv.axon_hooks  # noqa: F401
        return True
    except ImportError:
        pass
    try:
        import antenv
        from trn_agent_boot.trn_boot import _ntff_profile_via_ctypes
        mod = types.ModuleType("antenv.axon_hooks")
        state = {}
        mod.set_axon_ntff_profile_hook = lambda h: state.update(h=h)
        mod.get_axon_ntff_profile_hook = lambda: state.get("h")
        sys.modules["antenv.axon_hooks"] = mod
        antenv.axon_hooks = mod
        hook = _ntff_profile_via_ctypes("/opt/axon/libaxon_pjrt.so")
        mod.set_axon_ntff_profile_hook(hook)
        return hook is not None
    except Exception:
        return False


def _run(inputs, trace=False):
    from concourse.bass_utils import run_bass_kernel_spmd
    if trace:
        trace = _ensure_ntff_hook()
    if "nc" not in _PLAN:
        _PLAN["nc"] = _build_program()
    nc = _PLAN["nc"]
    in_maps = _prep_in_maps(inputs)
    res = run_bass_kernel_spmd(nc, in_maps, core_ids=list(range(NC_N)),
                               trace=trace)
    per_core = [res.results[c]["out_vec"] for c in range(NC_N)]
    return _combine(per_core), res


def kernel(**inputs) -> np.ndarray:
    out, _ = _run(inputs, trace=False)
    return out


# revision 27
# speedup vs baseline: 1.0960x; 1.0545x over previous
"""Trainium2 Bass kernel for nn_EmergentRiskMetrics.

Contract: kernel(**inputs) takes the FULL unsharded inputs (as produced by
setup_inputs()) and returns the FULL output (shape [8], float32).

Sharding: data-parallel over the time axis. Each of the 8 cores owns 1024
contiguous window starts (plus a 128-row halo) for the two rolling-window
correlation scans; the sign-concordance partial sum and cross-sectional
stds are computed on the owning core and combined as scalars on the host.
The full-T covariance (needed on-device for the eigenvalue iteration) is
replicated: every core re-computes X^T X from bf16 tiles of the full
sequence (~1 us of 128^3 bf16 matmuls + ~2 MB of DMA, overlapped with the
rolling phase). An AllReduce-based variant was measured at ~66 us of pure
collective latency for 64 KB on this runtime — replication is far cheaper.

DMA-trigger serialization dominates small-tensor staging, so the host
packs every fp32 constant (masks, identity, MLP weights, positions, ...)
into ONE [128,1024] tensor, and the bf16 bands + pre-transposed chunk
into ONE [128,1664] tensor; x_full lands via 8 big strided DMAs split
across the two HWDGE queues (sync + scalar).

Windowed sums are banded-matrix matmuls on the tensor engine (bands
pre-scaled by 1/sqrt(w) so the mean-correction q^2 term folds into the
V-mask), u = 1/std via reciprocal_approx_fast + Sqrt, and the whole
rolling path runs in bf16 (validated: all rolling-derived outputs have
orders-of-magnitude margin against bf16 noise; d2 >= 0 holds exactly
because S and P derive from the same bf16 x). V*mask+reduce is fused via
tensor_tensor_reduce.

Top eigenvalue: corr is squared 9 times in bf16 (fp32 PSUM accumulate);
traces at step 6 (normalization) and step 9 give lam = (T9*T6^8)^(1/512)
on the host.

Device outputs are per-core partial scalars; the host only gathers them
(sums partial sums, applies the final scalar clips/divides) to assemble
the 8 outputs.
"""

import numpy as np

T = 8192
A = 128
W20 = 20
W10 = 10
NC_N = 8
CHUNK = 1024            # window starts per core
XROWS = 1152            # rows of per-core chunk (9 x 128, incl. halo)
NBLK = XROWS // 128     # 9
R20 = 128 + W20 - 1     # 147
R10 = 128 + W10 - 1     # 137
N20 = T - W20           # 8172 rolling-20 windows
N10 = T - W10           # 8182 rolling-10 windows
OUT_SLOTS = 24
INV_OD = 1.0 / (A * (A - 1))
# rolling20 > 0.7 in corr units == raw quadratic sum > this
THRESH20 = 0.7 * (A * (A - 1)) + A
# static normalization for the eigenvalue squaring chain (~trace(corr^64));
# only needs to be within ~e+-80 of the true value for fp32/bf16 range
EIG_C = 4.0e6

S_COUNT20, S_HIST10, S_RECENT10, S_CSSUM, S_CSFIRST, S_CSLAST, \
    S_SUMCORR, S_SUMABS, S_TRACE, S_PASUM, S_PAMAX, S_SEV, S_SSQ, \
    S_T6, S_T9 = range(15)

# packed fp32 constant tensor column layout
CP_IDENT = 0
CP_M20 = 128                 # 275 cols
CP_M10 = CP_M20 + R20 + 128  # 403, 265 cols
CP_V20 = CP_M10 + R10 + 128  # 668
CP_H10 = CP_V20 + 8
CP_R10 = CP_H10 + 8
CP_W1A = CP_R10 + 8          # 692
CP_W1B = CP_W1A + 128        # 820
CP_B1 = CP_W1B + 128         # 948
CP_GAM = CP_B1 + 1
CP_BET = CP_GAM + 1
CP_W2 = CP_BET + 1           # 951, 64 cols
CP_B2 = CP_W2 + 64           # 1015
CP_W3 = CP_B2 + 1            # 1016, 3 cols
CP_B3 = CP_W3 + 3            # 1019
CP_OH2 = CP_B3 + 1
CP_OH127 = CP_OH2 + 1
CP_POS = CP_OH127 + 1
CP_XLAST = CP_POS + 1
CP_N = CP_XLAST + 1          # 1024

BP_B0 = 0
BP_B1 = 256
BP_XT = 512
BP_N = BP_XT + XROWS         # 1664

_PLAN = {}


def _build_masks():
    # 0/1 window-membership bands (the q^2 term is handled separately)
    m20 = np.zeros((128, R20), np.float32)
    m10 = np.zeros((128, R10), np.float32)
    for j in range(128):
        m20[j, j:j + W20] = 1.0
        m10[j, j:j + W10] = 1.0
    return m20, m10


def _build_bands():
    # bands0/1 [128 t, 256]: cols 0:128 window-20 (scaled 1/sqrt20),
    # cols 128:256 window-10 (scaled 1/sqrt10). S' = B0^T x_k + B1^T x_{k+1}
    b0 = np.zeros((128, 256), np.float32)
    b1 = np.zeros((128, 256), np.float32)
    s20 = 1.0 / np.sqrt(W20)
    s10 = 1.0 / np.sqrt(W10)
    for j in range(128):
        b0[j:min(128, j + W20), j] = s20
        if j + W20 > 128:
            b1[0:j + W20 - 128, j] = s20
        b0[j:min(128, j + W10), 128 + j] = s10
        if j + W10 > 128:
            b1[0:j + W10 - 128, 128 + j] = s10
    return b0, b1


def _core_masks(c):
    g = c * CHUNK + np.arange(CHUNK)
    valid20 = (g < N20).astype(np.float32)
    hist10 = (g < N10 - 5).astype(np.float32)
    recent10 = ((g >= N10 - 5) & (g < N10)).astype(np.float32)
    # device layout [128 partitions (j in chunk), 8 chunk-columns]
    return (np.ascontiguousarray(valid20.reshape(8, 128).T),
            np.ascontiguousarray(hist10.reshape(8, 128).T),
            np.ascontiguousarray(recent10.reshape(8, 128).T))


def _build_program():
    import os
    import concourse.bacc as bacc
    import concourse.tile as tile
    from concourse import mybir

    kbits = int(os.environ.get("KBITS", "63"))
    bigdma = int(os.environ.get("BIGDMA", "1"))
    # tensor_tensor_reduce hard-crashes the exec unit on this runtime
    use_ttr = int(os.environ.get("TTR", "0"))
    DO_ROLL = kbits & 1
    DO_CS = kbits & 2
    DO_COV = kbits & 4
    DO_EIG = kbits & 8
    DO_POS = kbits & 16
    DO_MLP = kbits & 32

    f32 = mybir.dt.float32
    bf16 = mybir.dt.bfloat16
    ALU = mybir.AluOpType
    ACT = mybir.ActivationFunctionType
    AX = mybir.AxisListType

    nc = bacc.Bacc("TRN2", target_bir_lowering=False, debug=False,
                   num_devices=NC_N)

    def din(name, shape, dt=f32):
        return nc.dram_tensor(name, shape, dt, kind="ExternalInput").ap()

    # partition-major layouts (host pre-permuted): col block i of x_full_pm
    # is x[i*128:(i+1)*128, :] with time-on-partitions — plain contiguous
    # DMAs with one descriptor per partition.
    x_full_pm = din("x_full_pm", [128, 64 * 128], bf16)
    xchunk_pm = din("xchunk_pm", [128, XROWS], bf16)
    cpack_in = din("cpack", [128, CP_N])
    bpack_in = din("bpack", [128, BP_N], bf16)
    out_d = nc.dram_tensor("out_vec", [1, OUT_SLOTS], f32,
                           kind="ExternalOutput").ap()

    with tile.TileContext(nc) as tc:
        with tc.tile_pool(name="const", bufs=1) as cst, \
             tc.tile_pool(name="persist", bufs=1) as per, \
             tc.tile_pool(name="sgs", bufs=3) as sgs, \
             tc.tile_pool(name="wrk", bufs=3) as wrk, \
             tc.tile_pool(name="small", bufs=6) as sml, \
             tc.tile_pool(name="ps", bufs=1, space="PSUM") as ps:

            psum_bufs = {"covq": 1, "band": 2, "zp": 2, "big": 1, "sc": 2}

            def psum(shape, tag):
                return ps.tile(shape, f32, tag=tag, name=tag,
                               bufs=psum_bufs[tag])

            # ---- packed loads: bpack/xchunk on sync, cpack on scalar ----
            bpk = cst.tile([128, BP_N], bf16, tag="bpk")
            nc.sync.dma_start(bpk[:], bpack_in[:, :])
            xck = per.tile([128, XROWS], bf16, tag="xck")
            nc.sync.dma_start(xck[:], xchunk_pm[:, :])
            cpk = cst.tile([128, CP_N], f32, tag="cpk")
            nc.scalar.dma_start(cpk[:], cpack_in[:, :])

            b0b = bpk[:, BP_B0:BP_B0 + 256]
            b1b = bpk[:, BP_B1:BP_B1 + 256]
            xTb = bpk[:, BP_XT:BP_XT + XROWS]
            xcbs = [xck[:, j * 128:(j + 1) * 128] for j in range(NBLK)]

            ident = cpk[:, CP_IDENT:CP_IDENT + 128]
            # dedicated mask tiles (vector TENSOR_TENSOR against a slice of
            # the wide packed tile crashed the exec unit)
            m20t = cst.tile([128, R20], f32, tag="m20t")
            nc.gpsimd.tensor_copy(m20t[:], cpk[:, CP_M20:CP_M20 + R20])
            m10t = cst.tile([128, R10], f32, tag="m10t")
            nc.gpsimd.tensor_copy(m10t[:], cpk[:, CP_M10:CP_M10 + R10])
            m20 = m20t[:]
            m10 = m10t[:]
            v20 = cpk[:, CP_V20:CP_V20 + 8]
            h10 = cpk[:, CP_H10:CP_H10 + 8]
            r10 = cpk[:, CP_R10:CP_R10 + 8]
            w1a = cpk[:, CP_W1A:CP_W1A + 128]
            w1b = cpk[:, CP_W1B:CP_W1B + 128]
            b1 = cpk[:, CP_B1:CP_B1 + 1]
            gam = cpk[:, CP_GAM:CP_GAM + 1]
            bet = cpk[:, CP_BET:CP_BET + 1]
            w2 = cpk[:, CP_W2:CP_W2 + 64]
            b2 = cpk[0:64, CP_B2:CP_B2 + 1]
            w3 = cpk[0:64, CP_W3:CP_W3 + 3]
            b3 = cpk[0:3, CP_B3:CP_B3 + 1]
            oh2 = cpk[0:3, CP_OH2:CP_OH2 + 1]
            oh127 = cpk[:, CP_OH127:CP_OH127 + 1]
            pos_sb = cpk[:, CP_POS:CP_POS + 1]
            xl = cpk[:, CP_XLAST:CP_XLAST + 1]

            ones = cst.tile([128, 1], f32, tag="ones")
            nc.vector.memset(ones[:], 1.0)
            onesb = cst.tile([128, 1], bf16, tag="onesb")
            nc.vector.memset(onesb[:], 1.0)
            ones_row = cst.tile([1, 128], f32, tag="ones_row")
            nc.vector.memset(ones_row[:], 1.0)

            out_sb = per.tile([1, OUT_SLOTS], f32, tag="out_sb")
            nc.vector.memset(out_sb[:], 0.0)

            def slot(i):
                return out_sb[:, i:i + 1]

            def part_sum(out_ap, vec_sb):
                # partition-axis sum on the (otherwise idle) gpsimd engine
                nc.gpsimd.tensor_reduce(out_ap, vec_sb, axis=AX.C,
                                        op=ALU.add)

            # ---- full x for replicated cov: 2 halves on the 2 HWDGE queues
            xfp = per.tile([128, 64 * 128], bf16, tag="xfp")
            if DO_COV:
                # 4 quarters alternating queues; cov chunk k consumes
                # quarter k, so the earliest-landing quarters go first
                for i in range(4):
                    eng = nc.sync if i % 2 == 0 else nc.scalar
                    eng.dma_start(xfp[:, i * 2048:(i + 1) * 2048],
                                  x_full_pm[:, i * 2048:(i + 1) * 2048])

            # ---- per-tile squares (gpsimd; reads SBUF only) ----
            xsqbs = []
            for j in range(NBLK):
                xsqb = per.tile([128, 128], bf16, tag="xsqb%d" % j)
                nc.gpsimd.tensor_mul(xsqb[:], xcbs[j], xcbs[j])
                xsqbs.append(xsqb)

            # ---- sharded sign concordance ----
            mq = psum([128, 128], "big")
            for i in range(8):
                sg = sgs.tile([128, 128], bf16, tag="sg")
                nc.scalar.activation(sg[:], xcbs[i], ACT.Sign)
                nc.tensor.matmul(mq[:], lhsT=sg[:], rhs=sg[:],
                                 start=(i == 0), stop=(i == 7),
                                 skip_group_check=True)
            mr = sml.tile([128, 1], f32, tag="mr")
            nc.vector.tensor_reduce(mr[:], mq[:], axis=AX.X, op=ALU.add)
            part_sum(slot(S_SSQ), mr[:])

            # ---- cross-sectional sums (independent; fills startup) ----
            if DO_CS:
                cs_s = per.tile([128, 8], f32, tag="cs_s")
                cs_q = per.tile([128, 8], f32, tag="cs_q")
                for b in range(8):
                    nc.vector.tensor_reduce(cs_s[:, b:b + 1], xcbs[b],
                                            axis=AX.X, op=ALU.add)
                    nc.vector.tensor_reduce(cs_q[:, b:b + 1], xsqbs[b][:],
                                            axis=AX.X, op=ALU.add)

            # ================= position diversity =================
            if DO_POS:
                pa = per.tile([128, 1], f32, tag="pa")
                nc.scalar.activation(pa[:], pos_sb, ACT.Abs)
                part_sum(slot(S_PASUM), pa[:])
                nc.gpsimd.tensor_reduce(slot(S_PAMAX), pa[:], axis=AX.C,
                                        op=ALU.max)

            # ================= herding MLP =================
            if DO_MLP:
                h1p = psum([128, 1], "sc")
                nc.tensor.matmul(h1p[:], lhsT=w1a, rhs=xl, start=True,
                                 stop=False, skip_group_check=True)
                nc.tensor.matmul(h1p[:], lhsT=w1b, rhs=pos_sb,
                                 start=False, stop=True,
                                 skip_group_check=True)
                h1 = sml.tile([128, 1], f32, tag="h1")
                nc.scalar.activation(h1[:], h1p[:], ACT.Relu, bias=b1)
                gk = sml.tile([128, 1], f32, tag="gk")
                nc.vector.tensor_scalar(gk[:], gam,
                                        float(1.0 / np.sqrt(1.0 + 1e-5)),
                                        None, ALU.mult)
                h1b = sml.tile([128, 1], f32, tag="h1b")
                nc.vector.tensor_scalar(h1b[:], h1[:], gk[:], bet,
                                        ALU.mult, ALU.add)
                h2p = psum([64, 1], "sc")
                nc.tensor.matmul(h2p[:], lhsT=w2, rhs=h1b[:], start=True,
                                 stop=True, skip_group_check=True)
                h2 = sml.tile([64, 1], f32, tag="h2")
                nc.scalar.activation(h2[:], h2p[:], ACT.Relu, bias=b2)
                lg = psum([3, 1], "sc")
                nc.tensor.matmul(lg[:], lhsT=w3, rhs=h2[:], start=True,
                                 stop=True, skip_group_check=True)
                exps = sml.tile([3, 1], f32, tag="exps")
                nc.scalar.activation(exps[:], lg[:], ACT.Exp, bias=b3)
                esum_sb = sml.tile([1, 1], f32, tag="esum_sb")
                part_sum(esum_sb[:], exps[:])
                erec = sml.tile([1, 1], f32, tag="erec")
                nc.vector.reciprocal(erec[:], esum_sb[:])
                e2p = psum([1, 1], "sc")
                nc.tensor.matmul(e2p[:], lhsT=oh2, rhs=exps[:], start=True,
                                 stop=True, skip_group_check=True)
                e2_sb = sml.tile([1, 1], f32, tag="e2_sb")
                nc.vector.tensor_copy(e2_sb[:], e2p[:])
                nc.vector.tensor_mul(slot(S_SEV), e2_sb[:], erec[:])

            # ---- cov post + eig emitted as closures, woven into the loop ----
            eig_state = {}

            def cov_post():
                cov = per.tile([128, 128], f32, tag="cov")
                nc.scalar.activation(cov[:], covq[:], ACT.Copy)
                dscr = wrk.tile([128, 128], f32, tag="dscr")
                nc.vector.tensor_mul(dscr[:], cov[:], ident)
                diag = per.tile([128, 1], f32, tag="diag")
                nc.vector.tensor_reduce(diag[:], dscr[:], axis=AX.X,
                                        op=ALU.add)
                dstd = per.tile([128, 1], f32, tag="dstd")
                nc.scalar.activation(dstd[:], diag[:], ACT.Sqrt)
                ucol = per.tile([128, 1], f32, tag="ucol")
                nc.vector.reciprocal(ucol[:], dstd[:])
                u2 = sml.tile([128, 1], f32, tag="u2")
                nc.vector.tensor_mul(u2[:], ucol[:], ucol[:])
                du2 = sml.tile([128, 1], f32, tag="du2")
                nc.vector.tensor_mul(du2[:], u2[:], diag[:])
                part_sum(slot(S_TRACE), du2[:])

                uT_p = psum([1, 128], "sc")
                nc.tensor.transpose(uT_p[:], ucol[:], ident)
                uT = per.tile([1, 128], f32, tag="uT")
                nc.vector.tensor_copy(uT[:], uT_p[:])

                def quad_form(mat_sb, out_slot):
                    qr = psum([1, 128], "sc")
                    nc.tensor.matmul(qr[:], lhsT=ucol[:], rhs=mat_sb,
                                     start=True, stop=True,
                                     skip_group_check=True)
                    qscr = sml.tile([1, 128], f32, tag="qscr")
                    nc.vector.tensor_mul(qscr[:], qr[:], uT[:])
                    qacc = sml.tile([1, 1], f32, tag="qacc")
                    nc.vector.tensor_reduce(qacc[:], qscr[:], axis=AX.X,
                                            op=ALU.add)
                    nc.vector.tensor_copy(out_slot, qacc[:])

                quad_form(cov[:], slot(S_SUMCORR))
                acov = per.tile([128, 128], f32, tag="acov")
                nc.scalar.activation(acov[:], cov[:], ACT.Abs)
                quad_form(acov[:], slot(S_SUMABS))

                # corr = diag(u) cov diag(u) -> bf16
                brow = per.tile([128, 128], f32, tag="brow")
                nc.vector.tensor_scalar(brow[:], cov[:], ucol[:], None,
                                        ALU.mult)
                bt_p = psum([128, 128], "big")
                nc.tensor.transpose(bt_p[:], brow[:], ident)
                corr = per.tile([128, 128], bf16, tag="corr")
                nc.scalar.activation(corr[:], bt_p[:], ACT.Copy,
                                     scale=ucol[:])
                eig_state["M"] = corr

            def eig_steps(lo, hi):
                # squaring steps lo..hi-1; static 1/EIG_C normalization at
                # step 5; trace of corr^512 at step 8
                M = eig_state["M"]
                for kk in range(lo, hi):
                    p = psum([128, 128], "big")
                    nc.tensor.matmul(p[:], lhsT=M[:], rhs=M[:],
                                     start=True, stop=True,
                                     skip_group_check=True)
                    Mn = wrk.tile([128, 128], bf16, tag="Mn")
                    if kk == 8:
                        escr = wrk.tile([128, 128], f32, tag="escr")
                        nc.vector.tensor_mul(escr[:], p[:], ident)
                        edg = sml.tile([128, 1], f32, tag="edg")
                        nc.vector.tensor_reduce(edg[:], escr[:], axis=AX.X,
                                                op=ALU.add)
                        part_sum(slot(S_T9), edg[:])
                        break
                    nc.scalar.activation(Mn[:], p[:], ACT.Copy,
                                         scale=(1.0 / EIG_C if kk == 5
                                                else 1.0))
                    M = Mn
                eig_state["M"] = M

            # ====== rolling windows + cov + eig chain, interleaved ======
            covq = psum([128, 128], "covq")
            num20 = per.tile([128, 8], f32, tag="num20")
            num10 = per.tile([128, 8], f32, tag="num10")
            qsq20 = per.tile([128, 8], f32, tag="qsq20")
            qsq10 = per.tile([128, 8], f32, tag="qsq10")
            for k in range(8):
                if DO_ROLL:
                    sp_ = psum([128, 256], "band")
                    nc.tensor.matmul(sp_[:], lhsT=xcbs[k], rhs=b0b,
                                     start=True, stop=False,
                                     skip_group_check=True)
                    nc.tensor.matmul(sp_[:], lhsT=xcbs[k + 1], rhs=b1b,
                                     start=False, stop=True,
                                     skip_group_check=True)
                    pp = psum([128, 256], "band")
                    nc.tensor.matmul(pp[:], lhsT=xsqbs[k][:], rhs=b0b,
                                     start=True, stop=False,
                                     skip_group_check=True)
                    nc.tensor.matmul(pp[:], lhsT=xsqbs[k + 1][:], rhs=b1b,
                                     start=False, stop=True,
                                     skip_group_check=True)
                    # d2 = P - S^2/w = pp*sqrt(w) - S'^2 (>=0 exactly)
                    spc = wrk.tile([128, 256], f32, tag="spc")
                    nc.vector.tensor_copy(spc[:], sp_[:])
                    sq = wrk.tile([128, 256], f32, tag="sq")
                    nc.gpsimd.tensor_mul(sq[:], spc[:], spc[:])
                    d2 = wrk.tile([128, 256], f32, tag="d2")
                    nc.vector.scalar_tensor_tensor(
                        d2[:, 0:128], in0=pp[:, 0:128],
                        scalar=float(np.sqrt(W20)),
                        in1=sq[:, 0:128], op0=ALU.mult, op1=ALU.subtract)
                    nc.vector.scalar_tensor_tensor(
                        d2[:, 128:256], in0=pp[:, 128:256],
                        scalar=float(np.sqrt(W10)),
                        in1=sq[:, 128:256], op0=ALU.mult, op1=ALU.subtract)
                    rd2 = wrk.tile([128, 256], f32, tag="rd2")
                    nc.vector.reciprocal_approx_fast(rd2[:], d2[:])
                    ub = wrk.tile([128, 256], bf16, tag="ub")
                    nc.scalar.activation(ub[:], rd2[:], ACT.Sqrt)

                    # q_j = sum_a u*S' via elementwise mul + ones-matmul
                    us = wrk.tile([128, 256], bf16, tag="us")
                    nc.gpsimd.tensor_mul(us[:], ub[:], spc[:])
                    for (wi, R, msk) in ((0, R20, m20), (1, R10, m10)):
                        qp = psum([128, 1], "sc")
                        nc.tensor.matmul(
                            qp[:], lhsT=us[:, wi * 128:(wi + 1) * 128],
                            rhs=onesb[:], start=True, stop=True,
                            skip_group_check=True)
                        qdst = qsq20 if wi == 0 else qsq10
                        qc = sml.tile([128, 1], f32, tag="qc")
                        nc.vector.tensor_copy(qc[:], qp[:])
                        nc.gpsimd.tensor_mul(qdst[:, k:k + 1], qc[:], qc[:])
                        zp = psum([128, R], "zp")
                        nc.tensor.matmul(
                            zp[:], lhsT=ub[:, wi * 128:(wi + 1) * 128],
                            rhs=xTb[:, k * 128:k * 128 + R],
                            start=True, stop=True, skip_group_check=True)
                        # mask to the window band, then square+row-reduce in
                        # one scalar-engine activation (accum_out)
                        zm = wrk.tile([128, R], bf16, tag="zm%d" % wi)
                        nc.vector.tensor_mul(zm[:], zp[:], msk)
                        V = wrk.tile([128, R], bf16, tag="V%d" % wi)
                        dst = num20 if wi == 0 else num10
                        nc.scalar.activation(V[:], zm[:], ACT.Square,
                                             accum_out=dst[:, k:k + 1])
                # front-load the 64 replicated cov matmuls into chunks 0-4
                # so the eigenvalue chain can start while rolling finishes
                if DO_COV and k < 5:
                    lo = [0, 13, 26, 39, 52][k]
                    hi = [13, 26, 39, 52, 64][k]
                    for i in range(lo, hi):
                        t_ = xfp[:, i * 128:(i + 1) * 128]
                        nc.tensor.matmul(covq[:], lhsT=t_, rhs=t_,
                                         start=(i == 0), stop=(i == 63),
                                         skip_group_check=True)
                if DO_COV and k == 5:
                    cov_post()
                if DO_COV and DO_EIG and k >= 6:
                    eig_steps(4 * (k - 6), 4 * (k - 5))
            if DO_COV and DO_EIG:
                eig_steps(8, 9)

            if DO_ROLL:
                # roll quadratic sums: n = sum(zm^2) - q^2, batched [128,8]
                n20 = sml.tile([128, 8], f32, tag="n20")
                nc.vector.tensor_tensor(n20[:], num20[:], qsq20[:],
                                        op=ALU.subtract)
                n10 = sml.tile([128, 8], f32, tag="n10")
                nc.vector.tensor_tensor(n10[:], num10[:], qsq10[:],
                                        op=ALU.subtract)
                # phase locking count: n20 > thresh, masked valid
                cmp = sml.tile([128, 8], f32, tag="cmp")
                nc.vector.tensor_scalar(cmp[:], n20[:], THRESH20, None,
                                        ALU.is_gt)
                cmp2 = sml.tile([128, 8], f32, tag="cmp2")
                nc.gpsimd.tensor_mul(cmp2[:], cmp[:], v20)
                cnt = sml.tile([128, 1], f32, tag="cnt")
                nc.vector.tensor_reduce(cnt[:], cmp2[:], axis=AX.X,
                                        op=ALU.add)
                part_sum(slot(S_COUNT20), cnt[:])
                hv = sml.tile([128, 8], f32, tag="hv")
                nc.gpsimd.tensor_mul(hv[:], n10[:], h10)
                hs = sml.tile([128, 1], f32, tag="hs")
                nc.vector.tensor_reduce(hs[:], hv[:], axis=AX.X, op=ALU.add)
                part_sum(slot(S_HIST10), hs[:])
                rv = sml.tile([128, 8], f32, tag="rv")
                nc.gpsimd.tensor_mul(rv[:], n10[:], r10)
                rs = sml.tile([128, 1], f32, tag="rs")
                nc.vector.tensor_reduce(rs[:], rv[:], axis=AX.X, op=ALU.add)
                part_sum(slot(S_RECENT10), rs[:])

            # ---- cross-sectional std finish ----
            if DO_CS:
                cs_sq = sml.tile([128, 8], f32, tag="cs_sq")
                nc.scalar.activation(cs_sq[:], cs_s[:], ACT.Square)
                cs_var = sml.tile([128, 8], f32, tag="cs_var")
                nc.vector.scalar_tensor_tensor(
                    cs_var[:], in0=cs_sq[:], scalar=-1.0 / A, in1=cs_q[:],
                    op0=ALU.mult, op1=ALU.add)
                csstd = per.tile([128, 8], f32, tag="csstd")
                nc.scalar.activation(csstd[:], cs_var[:], ACT.Sqrt,
                                     scale=1.0 / (A - 1))
                csr = sml.tile([128, 1], f32, tag="csr")
                nc.vector.tensor_reduce(csr[:], csstd[:], axis=AX.X,
                                        op=ALU.add)
                part_sum(slot(S_CSSUM), csr[:])
                nc.vector.tensor_copy(slot(S_CSFIRST), csstd[0:1, 0:1])
                cslast_p = psum([1, 1], "sc")
                nc.tensor.matmul(cslast_p[:], lhsT=oh127, rhs=csstd[:, 7:8],
                                 start=True, stop=True, skip_group_check=True)
                nc.vector.tensor_copy(slot(S_CSLAST), cslast_p[:])

            # ================= write out =================
            nc.sync.dma_start(out_d[:, :], out_sb[:])

    nc.compile()
    return nc


def _prep_in_maps(inputs):
    import ml_dtypes
    bfloat16 = ml_dtypes.bfloat16
    x = np.ascontiguousarray(np.asarray(inputs["returns_sequence"],
                                        dtype=np.float32))
    xb = x.astype(bfloat16)
    m20, m10 = _build_masks()
    b0, b1 = _build_bands()

    cpack = np.zeros((128, CP_N), np.float32)
    cpack[:, CP_IDENT:CP_IDENT + 128] = np.eye(128, dtype=np.float32)
    cpack[:, CP_M20:CP_M20 + R20] = m20
    cpack[:, CP_M10:CP_M10 + R10] = m10
    w1 = np.asarray(inputs["w1"], np.float32)
    cpack[:, CP_W1A:CP_W1A + 128] = w1[0:128]
    cpack[:, CP_W1B:CP_W1B + 128] = w1[128:256]
    cpack[:, CP_B1] = np.asarray(inputs["b1"], np.float32)
    cpack[:, CP_GAM] = np.asarray(inputs["gamma"], np.float32)
    cpack[:, CP_BET] = np.asarray(inputs["beta"], np.float32)
    cpack[:, CP_W2:CP_W2 + 64] = np.asarray(inputs["w2"], np.float32)
    cpack[0:64, CP_B2] = np.asarray(inputs["b2"], np.float32)
    cpack[0:64, CP_W3:CP_W3 + 3] = np.asarray(inputs["w3"], np.float32)
    cpack[0:3, CP_B3] = np.asarray(inputs["b3"], np.float32)
    cpack[2, CP_OH2] = 1.0
    cpack[127, CP_OH127] = 1.0
    cpack[:, CP_POS] = np.asarray(inputs["positions"], np.float32)
    cpack[:, CP_XLAST] = x[-1]

    # partition-major full x: col block i is rows [i*128,(i+1)*128)
    xfull_pm = np.ascontiguousarray(
        xb.reshape(64, 128, 128).transpose(1, 0, 2).reshape(128, 64 * 128))

    in_maps = []
    for c in range(NC_N):
        rows = (c * CHUNK + np.arange(XROWS)) % T
        v20, h10, r10 = _core_masks(c)
        cp = cpack.copy()
        cp[:, CP_V20:CP_V20 + 8] = v20
        cp[:, CP_H10:CP_H10 + 8] = h10
        cp[:, CP_R10:CP_R10 + 8] = r10
        xcb = np.ascontiguousarray(xb[rows])
        xchunk_pm = np.ascontiguousarray(
            xcb.reshape(NBLK, 128, 128).transpose(1, 0, 2)
            .reshape(128, XROWS))
        bpack = np.zeros((128, BP_N), bfloat16)
        bpack[:, BP_B0:BP_B0 + 256] = b0.astype(bfloat16)
        bpack[:, BP_B1:BP_B1 + 256] = b1.astype(bfloat16)
        bpack[:, BP_XT:BP_XT + XROWS] = xcb.T
        in_maps.append({
            "x_full_pm": xfull_pm,
            "xchunk_pm": xchunk_pm,
            "cpack": cp,
            "bpack": bpack,
        })
    return in_maps


def _combine(per_core):
    count20 = sum(float(per_core[c][0, S_COUNT20]) for c in range(NC_N))
    hist_raw = sum(float(per_core[c][0, S_HIST10]) for c in range(NC_N))
    rec_raw = sum(float(per_core[c][0, S_RECENT10]) for c in range(NC_N))
    cs_sum = sum(float(per_core[c][0, S_CSSUM]) for c in range(NC_N))
    ssq_sum = sum(float(per_core[c][0, S_SSQ]) for c in range(NC_N))
    cs_first = float(per_core[0][0, S_CSFIRST])
    cs_last = float(per_core[NC_N - 1][0, S_CSLAST])
    r0 = per_core[0][0]
    sum_corr = float(r0[S_SUMCORR])
    sum_abs = float(r0[S_SUMABS])
    trace_c = float(r0[S_TRACE])
    pa_sum = float(r0[S_PASUM])
    pa_max = float(r0[S_PAMAX])
    severity = float(r0[S_SEV])
    T9 = float(r0[S_T9])

    phase_locking = count20 / N20
    nh = N10 - 5
    hist = (hist_raw - nh * A) * INV_OD / nh
    recent = (rec_raw - 5 * A) * INV_OD / 5.0
    surge = 0.0
    if hist > 0:
        surge = min(max((recent - hist) / hist, 0.0), 1.0)
    avg_disp = cs_sum / T
    trend = -(cs_last - cs_first) / (T - 1)
    herding_index = min(max(trend / (avg_disp + 1e-6) + 0.5, 0.0), 1.0)
    avg_corr = (sum_corr - trace_c) / (A * (A - 1))
    # T9 = trace(corr^512) / EIG_C^8 on device; lam ~ trace(corr^512)^(1/512)
    lam = np.exp((8.0 * np.log(EIG_C) + np.log(T9)) / 512.0)
    sync_risk = min(1.0, (lam / A) * avg_corr)
    return_div = 1.0 - sum_abs / (A * A)
    pos_div = 1.0 - pa_max / pa_sum
    div_loss = 1.0 - np.sqrt(return_div * pos_div)
    avg_conc = (A * A / 2.0 + ssq_sum / (2.0 * T) - A) / (A * (A - 1))
    phase_coupling = min(max((avg_conc - 0.5) * 2.0, 0.0), 1.0)
    collective = (herding_index + sync_risk + div_loss) / 3.0
    return np.array([herding_index, severity, sync_risk, phase_locking,
                     div_loss, surge, phase_coupling, collective],
                    dtype=np.float32)


def _ensure_ntff_hook():
    """Install the axon NTFF profile hook if the image lacks antenv.axon_hooks."""
    import sys
    import types
    try:
        import antenv.axon_hooks  # noqa: F401
        return True
    except ImportError:
        pass
    try:
        import antenv
        from trn_agent_boot.trn_boot import _ntff_profile_via_ctypes
        mod = types.ModuleType("antenv.axon_hooks")
        state = {}
        mod.set_axon_ntff_profile_hook = lambda h: state.update(h=h)
        mod.get_axon_ntff_profile_hook = lambda: state.get("h")
        sys.modules["antenv.axon_hooks"] = mod
        antenv.axon_hooks = mod
        hook = _ntff_profile_via_ctypes("/opt/axon/libaxon_pjrt.so")
        mod.set_axon_ntff_profile_hook(hook)
        return hook is not None
    except Exception:
        return False


def _run(inputs, trace=False):
    from concourse.bass_utils import run_bass_kernel_spmd
    if trace:
        trace = _ensure_ntff_hook()
    if "nc" not in _PLAN:
        _PLAN["nc"] = _build_program()
    nc = _PLAN["nc"]
    in_maps = _prep_in_maps(inputs)
    res = run_bass_kernel_spmd(nc, in_maps, core_ids=list(range(NC_N)),
                               trace=trace)
    per_core = [res.results[c]["out_vec"] for c in range(NC_N)]
    return _combine(per_core), res


def kernel(**inputs) -> np.ndarray:
    out, _ = _run(inputs, trace=False)
    return out
